# revision 1
# baseline (speedup 1.0000x reference)
"""Trainium2 Bass kernel for nn_HashingModel (retrieval_knn).

Sharding: data-parallel over batch B across 8 cores (256 rows each).
Cross-batch MHA handled by AllGather of the per-shard K/V projections.
All heavy matmuls in bf16; similarity+argmax in fp32 (tie safety).
Activations flow transposed ([feature, batch] layouts) so weights can be
used pre-transposed (host-side layout prep) without on-device transposes.
Softmax skips max-subtraction (scores are tiny: weights scaled 0.02); the
denominator comes free from a ones-column appended to V.

Self-contained: hardcoded shapes, no file reads.
"""
import sys
import numpy as np

sys.path.insert(0, '/opt/trn_rl_repo')

import ml_dtypes
from concourse import bass, bacc, tile, mybir
from concourse.bass_utils import run_bass_kernel_spmd

dt = mybir.dt
BF16 = ml_dtypes.bfloat16
AF = mybir.ActivationFunctionType

FULL = dict(NC=8, B=2048, E=512, P=4096, H=8, HD=64, HID=4096, BIT=64)


def _cfg(NC, B, E, P, H, HD, HID, BIT):
    c = dict(NC=NC, B=B, E=E, P=P, H=H, HD=HD, HID=HID, BIT=BIT)
    c['BS'] = B // NC          # batch shard per core
    c['E2'] = 2 * E            # MLP input dim
    c['EPAD'] = ((E + 1 + 127) // 128) * 128  # padded prompt row (ones col at E)
    c['KT_E'] = E // 128       # k-tiles over E
    c['KT_E2'] = 2 * E // 128
    c['NT_HID'] = HID // 128
    c['LT'] = c['BS'] // 128   # l-tiles per shard
    c['ST'] = B // 128         # s-tiles over full batch
    c['PC'] = P // 512         # prompt chunks for sim
    c['B2'] = 2 * c['BS']      # MLP free dim (fi|ft)
    return c


def build_nc(cfg, n_cores):
    import os
    F_H1X = os.environ.get('F_H1X', '1') == '1'
    F_EXPPAIR = int(os.environ.get('F_EXPPAIR', '1'))
    C = cfg
    NC = n_cores
    E, P, H, HD, HID, BIT = C['E'], C['P'], C['H'], C['HD'], C['HID'], C['BIT']
    BS, E2, EPAD = C['BS'], C['E2'], C['EPAD']
    KT_E, KT_E2, NT_HID, LT, ST, PC, B2 = (C['KT_E'], C['KT_E2'], C['NT_HID'],
                                           C['LT'], C['ST'], C['PC'], C['B2'])
    HPT = 128 // HD            # heads per 128-partition tile (2)
    NHT = E // 128             # eo tiles (4)
    SEG = HD + 1               # vaug segment width (65)
    S = C['ST'] * 128          # full batch (attention keys)

    nc = bacc.Bacc("TRN2", target_bir_lowering=False, debug=False,
                   num_devices=NC)

    mods = ['i', 't']
    inp = {}

    def din(name, shape, d):
        inp[name] = nc.dram_tensor(name, shape, d, kind="ExternalInput")

    for m in mods:
        din(f'xT_{m}', [E, BS], dt.float32)
        for w in ['wqT', 'wkT', 'wvT', 'woT']:
            din(f'{w}_{m}', [E, E], dt.bfloat16)
        din(f'bq_{m}', [E], dt.float32)
        din(f'bo_{m}', [E], dt.float32)
        din(f'bk_{m}', [1, E], dt.bfloat16)
        din(f'bv_{m}', [1, E], dt.bfloat16)
    din('promptsT', [E, P], dt.float32)
    din('prompts_pad', [P, EPAD], dt.bfloat16)
    for M in ['img', 'txt']:
        din(f'w1T_{M}', [NT_HID, KT_E2, 128, 128], dt.bfloat16)
        din(f'w2T_{M}', [NT_HID, NT_HID, 128, 128], dt.bfloat16)
        din(f'wcT_{M}', [NT_HID, 128, BIT], dt.bfloat16)
        din(f'b1_{M}', [HID], dt.float32)
        din(f'b2_{M}', [HID], dt.float32)
        din(f'bc_{M}', [1, BIT], dt.bfloat16)

    outs = {}
    for name in ['image_hash', 'text_hash', 'distill_i', 'distill_t']:
        outs[name] = nc.dram_tensor(name, [BS, BIT], dt.float32,
                                    kind="ExternalOutput")

    idx_scr = {m: nc.dram_tensor(f'idx_scr_{m}', [BS], dt.uint32) for m in mods}
    h1x_dram = {M: nc.dram_tensor(f'h1x_{M}', [NT_HID, 128, B2], dt.bfloat16)
                for M in ['img', 'txt']}
    kv_len = 2 * E * BS        # kpT shard (E*BS) + vp shard (BS*E)
    kv_in = {m: nc.dram_tensor(f'kv_in_{m}', [kv_len], dt.bfloat16) for m in mods}
    kv_out = {m: nc.dram_tensor(f'kv_out_{m}', [NC * kv_len], dt.bfloat16,
                                addr_space="Shared") for m in mods}

    with tile.TileContext(nc) as tc:
        with tc.tile_pool(name="persist", bufs=1) as pp:
            xTbf = {m: pp.tile([128, KT_E, BS], dt.bfloat16, tag=f'xTbf{m}', name=f'xTbf{m}')
                    for m in mods}
            inT = pp.tile([128, KT_E2, B2], dt.bfloat16, tag='inT')
            ones_row = pp.tile([1, 128], dt.bfloat16, tag='ones')
            nc.vector.memset(ones_row[:], 1.0)
            rmT = {m: pp.tile([128, EPAD // 128, BS], dt.bfloat16, tag=f'rmT{m}', name=f'rmT{m}')
                   for m in mods}

            # ======== Phase A+B: per-modality sim -> argmax -> gather ->
            # K/V projection -> AllGather (AG of modality i overlaps the
            # similarity matmuls of modality t, keeping the PE dense) ========
            with (
                tc.tile_pool(name="phA", bufs=1) as ap_,
                tc.tile_pool(name="simbuf", bufs=2) as simp,
                tc.tile_pool(name="smA", bufs=2) as sp,
                tc.tile_pool(name="psA", bufs=3, space="PSUM") as psA,
                tc.tile_pool(name="smB", bufs=2) as spB,
                tc.tile_pool(name="psB1", bufs=2, space="PSUM") as psB1,
            ):
                xT32 = {m: ap_.tile([128, KT_E, BS], dt.float32, tag=f'xT32{m}', name=f'xT32{m}')
                        for m in mods}
                for mi, m in enumerate(mods):
                    nc.sync.dma_start(
                        xT32[m][:],
                        inp[f'xT_{m}'].ap().rearrange("(k p) b -> p k b", p=128))
                    nc.vector.tensor_copy(xTbf[m][:], xT32[m][:])
                    nc.vector.tensor_copy(inT[:, 0:KT_E, mi * BS:(mi + 1) * BS],
                                          xTbf[m][:])
                prT = ap_.tile([128, KT_E, P], dt.float32, tag='promptsT')
                prsrc = inp['promptsT'].ap().rearrange("(k p) n -> k p n", p=128)
                for k in range(KT_E):
                    nc.sync.dma_start(prT[:, k, :], prsrc[k])
                wkv = {}
                for m in mods:
                    wk = ap_.tile([128, KT_E, E], dt.bfloat16, tag=f'wk{m}', name=f'wk{m}')
                    wv = ap_.tile([128, KT_E, E], dt.bfloat16, tag=f'wv{m}', name=f'wv{m}')
                    nc.sync.dma_start(
                        wk[:], inp[f'wkT_{m}'].ap().rearrange("(k p) n -> p k n", p=128))
                    nc.sync.dma_start(
                        wv[:], inp[f'wvT_{m}'].ap().rearrange("(k p) n -> p k n", p=128))
                    bk = ap_.tile([1, E], dt.bfloat16, tag=f'bk{m}', name=f'bk{m}')
                    bv = ap_.tile([1, E], dt.bfloat16, tag=f'bv{m}', name=f'bv{m}')
                    nc.sync.dma_start(bk[:], inp[f'bk_{m}'].ap())
                    nc.sync.dma_start(bv[:], inp[f'bv_{m}'].ap())
                    wkv[m] = (wk, wv, bk, bv)

                for m in mods:
                    for lt in range(LT):
                        sim = simp.tile([128, P], dt.float32, tag='sim')
                        for pc in range(PC):
                            ps = psA.tile([128, 512], dt.float32, tag='ps_sim')
                            for k in range(KT_E):
                                nc.tensor.matmul(
                                    ps[:], xT32[m][:, k, lt * 128:(lt + 1) * 128],
                                    prT[:, k, pc * 512:(pc + 1) * 512],
                                    start=(k == 0), stop=(k == KT_E - 1))
                            nc.vector.tensor_copy(sim[:, pc * 512:(pc + 1) * 512],
                                                  ps[:])
                        m8 = sp.tile([128, 8], dt.float32, tag='m8')
                        i8 = sp.tile([128, 8], dt.uint32, tag='i8')
                        nc.vector.max(m8[:], sim[:])
                        nc.vector.max_index(i8[:], m8[:], sim[:])
                        nc.sync.dma_start(
                            idx_scr[m].ap()[lt * 128:(lt + 1) * 128], i8[:, 0:1])

                    # gather indices: idx16[j%16, j//16] = idx[j], replicated x8
                    FR = BS // 16
                    half = FR // LT
                    i32 = sp.tile([128, FR], dt.uint32, tag='i32g')
                    for a in range(2):
                        for lt in range(LT):
                            nc.sync.dma_start(
                                i32[16 * a:16 * (a + 1), lt * half:(lt + 1) * half],
                                idx_scr[m].ap()[lt * 128:(lt + 1) * 128]
                                .rearrange("(f p) -> p f", p=16))
                    for b in range(1, 4):
                        nc.vector.tensor_copy(i32[32 * b:32 * (b + 1), :],
                                              i32[0:32, :])
                    ix16 = sp.tile([128, FR], dt.int16, tag='i16g')
                    nc.vector.tensor_copy(ix16[:], i32[:])
                    # rmT[p, c, j] = prompts_pad[idx_j, c*128+p]
                    nc.gpsimd.dma_gather(
                        rmT[m][:], inp['prompts_pad'].ap(), ix16[:],
                        num_idxs=BS, num_idxs_reg=BS, elem_size=EPAD,
                        transpose=True)

                    wk, wv, bk, bv = wkv[m]
                    # kpT shard -> kv_in[0:E*BS] laid out [KT_E*128, BS]
                    for eo in range(NHT):
                        ps = psB1.tile([128, BS], dt.float32, tag='ps_kv')
                        for k in range(KT_E):
                            nc.tensor.matmul(
                                ps[:], wk[:, k, eo * 128:(eo + 1) * 128],
                                rmT[m][:, k, :], start=(k == 0), stop=False)
                        nc.tensor.matmul(
                            ps[:], bk[0:1, eo * 128:(eo + 1) * 128],
                            rmT[m][0:1, KT_E, :], start=False, stop=True)
                        kv_sb = spB.tile([128, BS], dt.bfloat16, tag='kvsb')
                        nc.vector.tensor_copy(kv_sb[:], ps[:])
                        nc.sync.dma_start(
                            kv_in[m].ap()[eo * 128 * BS:(eo + 1) * 128 * BS]
                            .rearrange("(p b) -> p b", p=128), kv_sb[:])
                    # vp shard -> kv_in[E*BS:] laid out [LT*128, E]
                    for st in range(LT):
                        ps = psB1.tile([128, E], dt.float32, tag='ps_kv')
                        for k in range(KT_E):
                            nc.tensor.matmul(
                                ps[:], rmT[m][:, k, st * 128:(st + 1) * 128],
                                wv[:, k, :], start=(k == 0), stop=False)
                        nc.tensor.matmul(
                            ps[:], rmT[m][0:1, KT_E, st * 128:(st + 1) * 128],
                            bv[:], start=False, stop=True)
                        kv_sb2 = spB.tile([128, E], dt.bfloat16, tag='kvsb')
                        nc.vector.tensor_copy(kv_sb2[:], ps[:])
                        nc.sync.dma_start(
                            kv_in[m].ap()[E * BS + st * 128 * E:
                                          E * BS + (st + 1) * 128 * E]
                            .rearrange("(p b) -> p b", p=128), kv_sb2[:])
                    nc.gpsimd.collective_compute(
                        "AllGather", mybir.AluOpType.bypass,
                        replica_groups=[list(range(NC))],
                        ins=[kv_in[m][:]], outs=[kv_out[m][:]])

                # h1x = W1[:, x-half] @ [xT_i | xT_t] + b1, precomputed into
                # DRAM while the AllGathers are in flight (fills the PE stall)
                for M in (['img', 'txt'] if F_H1X else []):
                    b1x = spB.tile([128, NT_HID], dt.float32, tag='b1x')
                    nc.sync.dma_start(
                        b1x[:], inp[f'b1_{M}'].ap().rearrange("(t p) -> p t", p=128))
                    for ht in range(NT_HID):
                        wblk = spB.tile([128, KT_E, 128], dt.bfloat16, tag='w1xblk')
                        nc.sync.dma_start(
                            wblk[:],
                            inp[f'w1T_{M}'].ap()[ht, 0:KT_E].rearrange("k p c -> p k c"))
                        ps = psB1.tile([128, B2], dt.float32, tag='ps_h1x')
                        for k in range(KT_E):
                            nc.tensor.matmul(ps[:], wblk[:, k, :], inT[:, k, :],
                                             start=(k == 0), stop=(k == KT_E - 1))
                        hx = spB.tile([128, B2], dt.bfloat16, tag='h1x_sb')
                        nc.vector.tensor_scalar_add(hx[:], ps[:], b1x[:, ht:ht + 1])
                        nc.sync.dma_start(h1x_dram[M].ap()[ht], hx[:])

            # ======== Phase C: MHA ========
            with (
                tc.tile_pool(name="phC", bufs=1) as cp_,
                tc.tile_pool(name="expp", bufs=3) as ep,
                tc.tile_pool(name="smC", bufs=2) as spC,
                tc.tile_pool(name="psO", bufs=1, space="PSUM") as psO,
                tc.tile_pool(name="psS", bufs=2, space="PSUM") as psS,
                tc.tile_pool(name="psE", bufs=1, space="PSUM") as psE,
            ):
                vaug = cp_.tile([128, ST, H * SEG], dt.bfloat16, tag='vaug')
                nc.vector.memset(vaug[:], 1.0)
                kpT = cp_.tile([128, NHT, S], dt.bfloat16, tag='kpT')

                for mi, m in enumerate(mods):
                    wq = cp_.tile([128, KT_E, E], dt.bfloat16, tag='wq')
                    wo = cp_.tile([128, KT_E, E], dt.bfloat16, tag='wo')
                    nc.sync.dma_start(
                        wq[:], inp[f'wqT_{m}'].ap().rearrange("(k p) n -> p k n", p=128))
                    nc.sync.dma_start(
                        wo[:], inp[f'woT_{m}'].ap().rearrange("(k p) n -> p k n", p=128))
                    bq = spC.tile([128, NHT], dt.float32, tag='bq')
                    bo = spC.tile([128, NHT], dt.float32, tag='bo')
                    nc.sync.dma_start(
                        bq[:], inp[f'bq_{m}'].ap().rearrange("(t p) -> p t", p=128))
                    nc.sync.dma_start(
                        bo[:], inp[f'bo_{m}'].ap().rearrange("(t p) -> p t", p=128))

                    qpT = cp_.tile([128, NHT, BS], dt.bfloat16, tag='qpT')
                    for eo in range(NHT):
                        ps = psE.tile([128, BS], dt.float32, tag='ps_q')
                        for k in range(KT_E):
                            nc.tensor.matmul(
                                ps[:], wq[:, k, eo * 128:(eo + 1) * 128],
                                xTbf[m][:, k, :], start=(k == 0),
                                stop=(k == KT_E - 1))
                        nc.vector.tensor_scalar_add(qpT[:, eo, :], ps[:],
                                                    bq[:, eo:eo + 1])

                    # K/V from AllGather: kpT[p, eo, r*BS+b] ; vaug data columns
                    ksrc = kv_out[m].ap().rearrange(
                        "(r q p b) -> q p r b", r=NC, q=2 * E // 128, p=128)
                    for eo in range(NHT):
                        nc.sync.dma_start(
                            kpT[:, eo, :].rearrange("p (r b) -> p r b", r=NC),
                            ksrc[eo])
                    vsrc = kv_out[m].ap().rearrange("(r x) -> r x", r=NC)
                    for st in range(ST):
                        r, hf = st // LT, st % LT
                        blk = vsrc[r][E * BS + hf * 128 * E:
                                      E * BS + (hf + 1) * 128 * E] \
                            .rearrange("(p h d) -> p h d", p=128, h=H)
                        nc.sync.dma_start(
                            vaug[:, st, :].rearrange("p (h s) -> p h s", h=H)
                            [:, :, 0:HD], blk)
                    pso = [psO.tile([SEG, HPT * BS], dt.float32, tag=f'pso{g}',
                                    name=f'pso{g}') for g in range(H // HPT)]
                    for st2 in range(0, ST, 2):
                        ex = ep.tile([128, H, 2 * BS], dt.bfloat16, tag='expT')
                        for g in range(H // HPT):
                            for hh in range(HPT):
                                h = g * HPT + hh
                                hb = hh * HD
                                if F_EXPPAIR >= 1:
                                    # two s-tiles share one psum bank: same PE
                                    # row-group -> sequential drains, one exp op
                                    pss = psS.tile([128, 2 * BS], dt.float32,
                                                   tag='ps_s')
                                    for sj in range(2):
                                        st = st2 + sj
                                        nc.tensor.matmul(
                                            pss[:, sj * BS:(sj + 1) * BS],
                                            kpT[hb:hb + HD, g,
                                                st * 128:(st + 1) * 128],
                                            qpT[hb:hb + HD, g, :],
                                            start=True, stop=True,
                                            skip_group_check=True)
                                    nc.scalar.activation(
                                        ex[:, h, :], pss[:], AF.Exp,
                                        bias=0.0, scale=float(1.0 / np.sqrt(HD)))
                                else:
                                    for sj in range(2):
                                        st = st2 + sj
                                        pss1 = psS.tile([128, BS], dt.float32,
                                                        tag='ps_s')
                                        nc.tensor.matmul(
                                            pss1[:],
                                            kpT[hb:hb + HD, g,
                                                st * 128:(st + 1) * 128],
                                            qpT[hb:hb + HD, g, :],
                                            start=True, stop=True)
                                        nc.scalar.activation(
                                            ex[:, h, sj * BS:(sj + 1) * BS],
                                            pss1[:], AF.Exp,
                                            bias=0.0,
                                            scale=float(1.0 / np.sqrt(HD)))
                                for sj in range(2):
                                    st = st2 + sj
                                    nc.tensor.matmul(
                                        pso[g][:, hh * BS:(hh + 1) * BS],
                                        vaug[:, st, h * SEG:(h + 1) * SEG],
                                        ex[:, h, sj * BS:(sj + 1) * BS],
                                        start=(st == 0), stop=(st == ST - 1),
                                        skip_group_check=True)
                    zr = spC.tile([1, H * BS], dt.float32, tag='zr')
                    for h in range(H):
                        nc.vector.reciprocal(
                            zr[0:1, h * BS:(h + 1) * BS],
                            pso[h // HPT][HD:HD + 1, (h % HPT) * BS:(h % HPT + 1) * BS])
                    zb = spC.tile([HD, H * BS], dt.float32, tag='zb')
                    nc.gpsimd.partition_broadcast(zb[:], zr[:])
                    aoT = cp_.tile([128, NHT, BS], dt.bfloat16, tag='aoT')
                    for h in range(H):
                        nc.vector.tensor_tensor(
                            out=aoT[(h % HPT) * HD:(h % HPT + 1) * HD, h // HPT, :],
                            in0=pso[h // HPT][0:HD, (h % HPT) * BS:(h % HPT + 1) * BS],
                            in1=zb[:, h * BS:(h + 1) * BS],
                            op=mybir.AluOpType.mult)

                    # enhT -> inT rows E..2E-1; x -> rows 0..E-1
                    for eo in range(NHT):
                        ps = psE.tile([128, BS], dt.float32, tag='ps_e')
                        for k in range(KT_E):
                            nc.tensor.matmul(
                                ps[:], wo[:, k, eo * 128:(eo + 1) * 128],
                                aoT[:, k, :], start=(k == 0), stop=(k == KT_E - 1))
                        nc.vector.tensor_scalar_add(
                            inT[:, KT_E + eo, mi * BS:(mi + 1) * BS], ps[:],
                            bo[:, eo:eo + 1])

            # ======== Phase D: the four MLPs (two weight passes) ========
            with (
                tc.tile_pool(name="phD", bufs=1) as dp_,
                tc.tile_pool(name="w1s", bufs=2) as wp,
                tc.tile_pool(name="w2s", bufs=3) as w2p,
                tc.tile_pool(name="smD", bufs=2) as spD,
                tc.tile_pool(name="psD", bufs=4, space="PSUM") as psD,
                tc.tile_pool(name="psD3", bufs=2, space="PSUM") as psD3,
            ):
                h1T = dp_.tile([128, NT_HID, B2], dt.bfloat16, tag='h1T')
                h2T = dp_.tile([128, NT_HID, B2], dt.bfloat16, tag='h2T')
                out_map = {'img': ['image_hash', 'distill_i'],
                           'txt': ['distill_t', 'text_hash']}
                for M in ['img', 'txt']:
                    b2 = spD.tile([128, NT_HID], dt.float32, tag='b2')
                    bc = spD.tile([1, BIT], dt.bfloat16, tag='bc')
                    nc.sync.dma_start(
                        b2[:], inp[f'b2_{M}'].ap().rearrange("(t p) -> p t", p=128))
                    nc.sync.dma_start(bc[:], inp[f'bc_{M}'].ap())

                    b1f = spD.tile([128, NT_HID], dt.float32, tag='b1f')
                    if not F_H1X:
                        nc.sync.dma_start(
                            b1f[:], inp[f'b1_{M}'].ap().rearrange("(t p) -> p t", p=128))
                    for ht in range(NT_HID):
                        if F_H1X:
                            wblk = wp.tile([128, KT_E, 128], dt.bfloat16, tag='w1blk')
                            nc.sync.dma_start(
                                wblk[:],
                                inp[f'w1T_{M}'].ap()[ht, KT_E:KT_E2]
                                .rearrange("k p c -> p k c"))
                            hx = wp.tile([128, B2], dt.bfloat16, tag='h1x_ld')
                            nc.sync.dma_start(hx[:], h1x_dram[M].ap()[ht])
                            ps = psD.tile([128, B2], dt.float32, tag='ps_h12')
                            for k in range(KT_E):
                                nc.tensor.matmul(ps[:], wblk[:, k, :],
                                                 inT[:, KT_E + k, :],
                                                 start=(k == 0), stop=(k == KT_E - 1))
                            hpre = wp.tile([128, B2], dt.float32, tag='h1pre')
                            nc.vector.tensor_tensor(out=hpre[:], in0=ps[:], in1=hx[:],
                                                    op=mybir.AluOpType.add)
                            nc.vector.tensor_scalar_max(h1T[:, ht, :], hpre[:], 0.0)
                        else:
                            wblk = wp.tile([128, KT_E2, 128], dt.bfloat16, tag='w1blk')
                            nc.sync.dma_start(
                                wblk[:],
                                inp[f'w1T_{M}'].ap()[ht].rearrange("k p c -> p k c"))
                            ps = psD.tile([128, B2], dt.float32, tag='ps_h12')
                            for k in range(KT_E2):
                                nc.tensor.matmul(ps[:], wblk[:, k, :], inT[:, k, :],
                                                 start=(k == 0), stop=(k == KT_E2 - 1))
                            nc.vector.tensor_scalar(
                                h1T[:, ht, :], ps[:], b1f[:, ht:ht + 1], 0.0,
                                op0=mybir.AluOpType.add, op1=mybir.AluOpType.max)

                    for ht in range(NT_HID):
                        wblk = w2p.tile([128, NT_HID, 128], dt.bfloat16, tag='w2blk')
                        nc.sync.dma_start(
                            wblk[:],
                            inp[f'w2T_{M}'].ap()[ht].rearrange("k p c -> p k c"))
                        ps = psD.tile([128, B2], dt.float32, tag='ps_h12')
                        for k in range(NT_HID):
                            nc.tensor.matmul(ps[:], wblk[:, k, :], h1T[:, k, :],
                                             start=(k == 0), stop=(k == NT_HID - 1))
                        nc.vector.tensor_scalar(
                            h2T[:, ht, :], ps[:], b2[:, ht:ht + 1], 0.0,
                            op0=mybir.AluOpType.add, op1=mybir.AluOpType.max)

                    wc = dp_.tile([128, NT_HID, BIT], dt.bfloat16, tag='wc')
                    nc.sync.dma_start(
                        wc[:], inp[f'wcT_{M}'].ap().rearrange("k p c -> p k c"))
                    for bci in range(B2 // 128):
                        ps = psD3.tile([128, BIT], dt.float32, tag='ps_h3')
                        for k in range(NT_HID):
                            nc.tensor.matmul(
                                ps[:], h2T[:, k, bci * 128:(bci + 1) * 128],
                                wc[:, k, :], start=(k == 0), stop=False)
                        nc.tensor.matmul(ps[:], ones_row[:], bc[:],
                                         start=False, stop=True)
                        sq = spD.tile([128, BIT], dt.float32, tag='sq')
                        ss = spD.tile([128, 1], dt.float32, tag='ss')
                        nc.scalar.activation(sq[:], ps[:], AF.Square,
                                             accum_out=ss[:])
                        rs = spD.tile([128, 1], dt.float32, tag='rs')
                        nc.vector.reciprocal(rs[:], ss[:])
                        rsq = spD.tile([128, 1], dt.float32, tag='rsq')
                        nc.scalar.sqrt(rsq[:], rs[:])
                        h3 = spD.tile([128, BIT], dt.float32, tag='h3')
                        nc.vector.tensor_scalar_mul(h3[:], ps[:], rsq[:])
                        oname = out_map[M][bci // LT]
                        row = (bci % LT) * 128
                        nc.sync.dma_start(outs[oname].ap()[row:row + 128, :], h3[:])

    nc.compile()
    return nc


def _prep_in_maps(cfg, n_cores, image_feature, text_feature, prompts,
                  img_in_w, img_in_b, img_out_w, img_out_b,
                  txt_in_w, txt_in_b, txt_out_w, txt_out_b,
                  img_W1, img_b1, img_W2, img_b2, img_Wc, img_bc,
                  txt_W1, txt_b1, txt_W2, txt_b2, txt_Wc, txt_bc):
    C = cfg
    E, P, BIT, BS = C['E'], C['P'], C['BIT'], C['BS']
    NT_HID, KT_E2 = C['NT_HID'], C['KT_E2']

    def bt(x):
        return np.ascontiguousarray(np.asarray(x).astype(BF16))

    common = {}
    common['promptsT'] = np.ascontiguousarray(prompts.T.astype(np.float32))
    pp_ = np.zeros((P, C['EPAD']), dtype=BF16)
    pp_[:, :E] = np.asarray(prompts).astype(BF16)
    pp_[:, E] = BF16(1.0)
    common['prompts_pad'] = pp_

    for m, in_w, in_b, out_w, out_b in [
            ('i', img_in_w, img_in_b, img_out_w, img_out_b),
            ('t', txt_in_w, txt_in_b, txt_out_w, txt_out_b)]:
        common[f'wqT_{m}'] = bt(in_w[:E].T)
        common[f'wkT_{m}'] = bt(in_w[E:2 * E].T)
        common[f'wvT_{m}'] = bt(in_w[2 * E:].T)
        common[f'woT_{m}'] = bt(out_w.T)
        common[f'bq_{m}'] = np.ascontiguousarray(in_b[:E].astype(np.float32))
        common[f'bk_{m}'] = bt(in_b[E:2 * E][None, :])
        common[f'bv_{m}'] = bt(in_b[2 * E:][None, :])
        common[f'bo_{m}'] = np.ascontiguousarray(out_b.astype(np.float32))

    for M, W1, b1, W2, b2, Wc, bc in [
            ('img', img_W1, img_b1, img_W2, img_b2, img_Wc, img_bc),
            ('txt', txt_W1, txt_b1, txt_W2, txt_b2, txt_Wc, txt_bc)]:
        w1t = np.asarray(W1).T.astype(BF16)      # [2E, HID]
        common[f'w1T_{M}'] = np.ascontiguousarray(
            w1t.reshape(KT_E2, 128, NT_HID, 128).transpose(2, 0, 1, 3))
        w2t = np.asarray(W2).T.astype(BF16)      # [HID, HID]
        common[f'w2T_{M}'] = np.ascontiguousarray(
            w2t.reshape(NT_HID, 128, NT_HID, 128).transpose(2, 0, 1, 3))
        wct = np.asarray(Wc).T.astype(BF16)      # [HID, BIT]
        common[f'wcT_{M}'] = np.ascontiguousarray(wct.reshape(NT_HID, 128, BIT))
        common[f'b1_{M}'] = np.ascontiguousarray(b1.astype(np.float32))
        common[f'b2_{M}'] = np.ascontiguousarray(b2.astype(np.float32))
        common[f'bc_{M}'] = bt(np.asarray(bc)[None, :])

    xTi = np.asarray(image_feature).T.astype(np.float32)
    xTt = np.asarray(text_feature).T.astype(np.float32)
    in_maps = []
    for c in range(n_cores):
        im = dict(common)
        im['xT_i'] = np.ascontiguousarray(xTi[:, c * BS:(c + 1) * BS])
        im['xT_t'] = np.ascontiguousarray(xTt[:, c * BS:(c + 1) * BS])
        in_maps.append(im)
    return in_maps


_NC_CACHE = {}


def _get_nc(cfg, n_cores):
    key = (tuple(sorted(cfg.items())), n_cores)
    if key not in _NC_CACHE:
        _NC_CACHE[key] = build_nc(cfg, n_cores)
    return _NC_CACHE[key]


def run(inputs, cfg=None, n_cores=None, trace=False):
    cfg = cfg or _cfg(**FULL)
    n_cores = n_cores or cfg['NC']
    nc = _get_nc(cfg, n_cores)
    in_maps = _prep_in_maps(cfg, n_cores, **{
        k: np.asarray(v) for k, v in inputs.items() if k != 'iteration'})
    res = run_bass_kernel_spmd(nc, in_maps, list(range(n_cores)), trace=trace)
    out = {}
    for name in ['image_hash', 'text_hash', 'distill_i', 'distill_t']:
        out[name] = np.concatenate(
            [res.results[c][name] for c in range(n_cores)], axis=0)
    return (out['image_hash'], out['text_hash'],
            out['distill_i'], out['distill_t']), res


def kernel(**inputs):
    (ih, th, di, dtl), _ = run(inputs)
    return ih, th, di, dtl



# revision 17
# speedup vs baseline: 1.0434x; 1.0434x over previous
"""Trainium2 Bass kernel for nn_HashingModel (retrieval_knn).

Sharding: data-parallel over batch B across 8 cores (256 rows each).

v2 design (vs v1's K/V AllGather):
- Cross-batch MHA needs K/V for all 2048 keys. Every core already holds the
  full prompt table in DRAM, so we AllGather only the argmax *indices*
  (1 KB vs 512 KB), gather all 2048 prompt rows locally, and project K/V
  for the full batch on every core. The extra ~30us of replicated matmul
  per modality replaces 47-84us AllGather stalls that also tripped the
  PE activity throttle (HAM K=4/8) for whole phases.
- Similarity+argmax in fp32 (argmax must match the reference exactly;
  fp32 matmul measured at 2 cyc/row on HW). Sim drains on the Scalar
  engine so the Vector engine is free for argmax.
- K-proj bias dropped (softmax row-shift invariant; zero in practice),
  V-proj bias folded into the value table drain.
- MHA score/exp/AV loop is exp(Scalar)-paced; PE bubbles are filled by
  interleaving the other modality's K/V projection and W1*x precompute.
- Final Wc layer runs transposed (weights stationary, batch moving,
  N=512 instead of N=64) + PE transpose before the l2norm chain.
- All heavy matmuls bf16; weights pre-transposed host-side.

Self-contained: hardcoded shapes, no file reads.
"""
import sys
import numpy as np

sys.path.insert(0, '/opt/trn_rl_repo')

import ml_dtypes
from concourse import bass, bacc, tile, mybir
from concourse.bass_utils import run_bass_kernel_spmd
from concourse.masks import make_identity

dt = mybir.dt
BF16 = ml_dtypes.bfloat16
AF = mybir.ActivationFunctionType

FULL = dict(NC=8, B=2048, E=512, P=4096, H=8, HD=64, HID=4096, BIT=64)


def _cfg(NC, B, E, P, H, HD, HID, BIT):
    c = dict(NC=NC, B=B, E=E, P=P, H=H, HD=HD, HID=HID, BIT=BIT)
    c['BS'] = B // NC          # batch shard per core
    c['E2'] = 2 * E            # MLP input dim
    c['KT_E'] = E // 128       # k-tiles over E
    c['KT_E2'] = 2 * E // 128
    c['NT_HID'] = HID // 128
    c['LT'] = c['BS'] // 128   # l-tiles per shard
    c['ST'] = B // 128         # s-tiles over full batch
    c['PC'] = P // 512         # prompt chunks for sim
    c['B2'] = 2 * c['BS']      # MLP free dim (fi|ft)
    return c


def build_nc(cfg, n_cores):
    import os
    F_FILL = os.environ.get('F_FILL', '1') == '1'
    C = cfg
    NC = n_cores
    E, P, H, HD, HID, BIT = C['E'], C['P'], C['H'], C['HD'], C['HID'], C['BIT']
    BS, E2 = C['BS'], C['E2']
    KT_E, KT_E2, NT_HID, LT, ST, PC, B2 = (C['KT_E'], C['KT_E2'], C['NT_HID'],
                                           C['LT'], C['ST'], C['PC'], C['B2'])
    HPT = 128 // HD            # heads per 128-partition tile (2)
    NHT = E // 128             # eo tiles (4)
    SEG = HD + 1               # vaug segment width (65)
    S = C['B']                 # full batch (attention keys)
    SC = S // 512              # 512-col chunks of S
    FR = S // 16               # wrap16 columns for gather indices

    nc = bacc.Bacc("TRN2", target_bir_lowering=False, debug=False,
                   num_devices=NC)

    mods = ['i', 't']
    inp = {}

    def din(name, shape, d):
        inp[name] = nc.dram_tensor(name, shape, d, kind="ExternalInput")

    for m in mods:
        din(f'xT_{m}', [E, BS], dt.float32)
        for w in ['wqT', 'wkT', 'wvT', 'woT']:
            din(f'{w}_{m}', [E, E], dt.bfloat16)
        din(f'bq_{m}', [E], dt.float32)
        din(f'bo_{m}', [E], dt.float32)
        din(f'bv_{m}', [1, E], dt.bfloat16)
    din('promptsT', [E, P], dt.float32)
    din('prompts_bf', [P, E], dt.bfloat16)
    for M in ['img', 'txt']:
        din(f'w1T_{M}', [NT_HID, KT_E2, 128, 128], dt.bfloat16)
        din(f'w2T_{M}', [NT_HID, NT_HID, 128, 128], dt.bfloat16)
        din(f'wcT_{M}', [NT_HID, 128, BIT], dt.bfloat16)
        din(f'b1_{M}', [HID], dt.float32)
        din(f'b2_{M}', [HID], dt.float32)
        din(f'bc_{M}', [1, BIT], dt.bfloat16)

    outs = {}
    for name in ['image_hash', 'text_hash', 'distill_i', 'distill_t']:
        outs[name] = nc.dram_tensor(name, [BS, BIT], dt.float32,
                                    kind="ExternalOutput")

    idx_in = {m: nc.dram_tensor(f'idx_in_{m}', [BS], dt.uint32) for m in mods}
    idx_out = {m: nc.dram_tensor(f'idx_out_{m}', [NC * BS], dt.uint32,
                                 addr_space="Shared") for m in mods}
    h1x_dram = {M: nc.dram_tensor(f'h1x_{M}', [NT_HID, 128, B2], dt.bfloat16)
                for M in ['img', 'txt']}

    with tile.TileContext(nc) as tc:
        with tc.tile_pool(name="persist", bufs=1) as pp:
            xTbf = {m: pp.tile([128, KT_E, BS], dt.bfloat16, tag=f'xTbf{m}',
                               name=f'xTbf{m}') for m in mods}
            inT = pp.tile([128, KT_E2, B2], dt.bfloat16, tag='inT')
            ones512 = pp.tile([1, 512], dt.bfloat16, tag='ones')
            nc.vector.memset(ones512[:], 1.0)
            ident = pp.tile([128, 128], dt.float32, tag='ident')
            make_identity(nc, ident[:])

            # weights + small tensors that live through phases A-C
            with tc.tile_pool(name="wts", bufs=1) as wt:
                wk, wv, wq, wo, bvb, bqc, boc = {}, {}, {}, {}, {}, {}, {}
                for m in mods:
                    wk[m] = wt.tile([128, KT_E, E], dt.bfloat16, tag=f'wk{m}',
                                    name=f'wk{m}')
                    wv[m] = wt.tile([128, KT_E, E], dt.bfloat16, tag=f'wv{m}',
                                    name=f'wv{m}')
                    wq[m] = wt.tile([128, KT_E, E], dt.bfloat16, tag=f'wq{m}',
                                    name=f'wq{m}')
                    wo[m] = wt.tile([128, KT_E, E], dt.bfloat16, tag=f'wo{m}',
                                    name=f'wo{m}')
                    nc.sync.dma_start(
                        wk[m][:],
                        inp[f'wkT_{m}'].ap().rearrange("(k p) n -> p k n", p=128))
                    nc.sync.dma_start(
                        wv[m][:],
                        inp[f'wvT_{m}'].ap().rearrange("(k p) n -> p k n", p=128))
                    nc.sync.dma_start(
                        wq[m][:],
                        inp[f'wqT_{m}'].ap().rearrange("(k p) n -> p k n", p=128))
                    nc.sync.dma_start(
                        wo[m][:],
                        inp[f'woT_{m}'].ap().rearrange("(k p) n -> p k n", p=128))
                    bvr = wt.tile([1, E], dt.bfloat16, tag=f'bvr{m}',
                                  name=f'bvr{m}')
                    nc.sync.dma_start(bvr[:], inp[f'bv_{m}'].ap())
                    bvb[m] = wt.tile([128, E], dt.bfloat16, tag=f'bvb{m}',
                                     name=f'bvb{m}')
                    nc.gpsimd.partition_broadcast(bvb[m][:], bvr[:])
                    bqc[m] = wt.tile([128, NHT], dt.float32, tag=f'bq{m}',
                                     name=f'bq{m}')
                    boc[m] = wt.tile([128, NHT], dt.float32, tag=f'bo{m}',
                                     name=f'bo{m}')
                    nc.sync.dma_start(
                        bqc[m][:],
                        inp[f'bq_{m}'].ap().rearrange("(t p) -> p t", p=128))
                    nc.sync.dma_start(
                        boc[m][:],
                        inp[f'bo_{m}'].ap().rearrange("(t p) -> p t", p=128))
                # gathered prompt rows, chunked so each 256-idx sub-gather
                # writes a contiguous [128, KT_E, 256] block
                JC = S // 256
                rmT = {m: wt.tile([128, JC, KT_E, 256], dt.bfloat16,
                                  tag=f'rmT{m}', name=f'rmT{m}') for m in mods}

                # ======== Phase A: sim -> argmax -> idx AllGather ->
                # local gather of all 2048 prompt rows per modality ========
                with (
                    tc.tile_pool(name="phA", bufs=1) as ap_,
                    tc.tile_pool(name="simbuf", bufs=2) as simp,
                    tc.tile_pool(name="smA", bufs=2) as sp,
                    tc.tile_pool(name="psA", bufs=4, space="PSUM") as psA,
                ):
                    xT32 = {m: ap_.tile([128, KT_E, BS], dt.float32,
                                        tag=f'xT32{m}', name=f'xT32{m}')
                            for m in mods}
                    for mi, m in enumerate(mods):
                        nc.sync.dma_start(
                            xT32[m][:],
                            inp[f'xT_{m}'].ap().rearrange("(k p) b -> p k b", p=128))
                        nc.vector.tensor_copy(xTbf[m][:], xT32[m][:])
                        nc.vector.tensor_copy(
                            inT[:, 0:KT_E, mi * BS:(mi + 1) * BS], xTbf[m][:])
                    prT = ap_.tile([128, KT_E, P], dt.float32, tag='promptsT')
                    prsrc = inp['promptsT'].ap().rearrange(
                        "(k p) (h n) -> k p h n", p=128, h=2)
                    for hf in range(2):
                        for k in range(KT_E):
                            nc.sync.dma_start(
                                prT[:, k, hf * (P // 2):(hf + 1) * (P // 2)],
                                prsrc[k, :, hf])

                    for m in mods:
                        sims = []
                        for lt in range(LT):
                            sim = simp.tile([128, P], dt.float32, tag='sim')
                            sims.append(sim)
                            for pc in range(PC):
                                ps = psA.tile([128, 512], dt.float32,
                                              tag='ps_sim')
                                for k in range(KT_E):
                                    nc.tensor.matmul(
                                        ps[:],
                                        xT32[m][:, k, lt * 128:(lt + 1) * 128],
                                        prT[:, k, pc * 512:(pc + 1) * 512],
                                        start=(k == 0), stop=(k == KT_E - 1))
                                # drain on Scalar engine: DVE stays free for
                                # argmax, PE for sim
                                nc.scalar.copy(sim[:, pc * 512:(pc + 1) * 512],
                                               ps[:])
                            m8 = sp.tile([128, 8], dt.float32, tag='m8')
                            i8 = sp.tile([128, 8], dt.uint32, tag='i8')
                            nc.vector.max(m8[:], sim[:])
                            nc.vector.max_index(i8[:], m8[:], sim[:])
                            nc.sync.dma_start(
                                idx_in[m].ap()[lt * 128:(lt + 1) * 128],
                                i8[:, 0:1])
                        nc.gpsimd.collective_compute(
                            "AllGather", mybir.AluOpType.bypass,
                            replica_groups=[list(range(NC))],
                            ins=[idx_in[m][:]], outs=[idx_out[m][:]])
                        # wrap16 + replicate + int16 for the gpsimd gather
                        i32 = sp.tile([128, FR], dt.uint32, tag='i32g')
                        for a in range(2):
                            nc.sync.dma_start(
                                i32[16 * a:16 * (a + 1), :],
                                idx_out[m].ap().rearrange("(f p) -> p f", p=16))
                        nc.vector.tensor_copy(i32[32:64, :], i32[0:32, :])
                        nc.vector.tensor_copy(i32[64:128, :], i32[0:64, :])
                        ix16 = sp.tile([128, FR], dt.int16, tag='i16g')
                        nc.vector.tensor_copy(ix16[:], i32[:])
                        for j in range(S // 256):
                            nc.gpsimd.dma_gather(
                                rmT[m][:, j], inp['prompts_bf'].ap(),
                                ix16[:, 16 * j:16 * (j + 1)],
                                num_idxs=256, num_idxs_reg=256, elem_size=E,
                                transpose=True)

                # ======== Phase C: K/V proj (full batch, local) + MHA.
                # The other modality's K/V projection and the W1*x
                # precompute fill the exp-paced PE bubbles. ========
                with (
                    tc.tile_pool(name="phC", bufs=1) as cp_,
                    tc.tile_pool(name="w1xp", bufs=3) as w1p,
                    tc.tile_pool(name="expp", bufs=2) as ep,
                    tc.tile_pool(name="smC", bufs=2) as spC,
                    tc.tile_pool(name="smZ", bufs=1) as spZ,
                    tc.tile_pool(name="psB", bufs=2, space="PSUM") as psB,
                    tc.tile_pool(name="psS", bufs=2, space="PSUM") as psS,
                    tc.tile_pool(name="psO", bufs=1, space="PSUM") as psO,
                ):
                    kpT = {m: cp_.tile([128, NHT, S], dt.bfloat16,
                                       tag=f'kpT{m}', name=f'kpT{m}')
                           for m in mods}
                    vaug = {m: cp_.tile([128, ST, H * SEG], dt.bfloat16,
                                        tag=f'vaug{m}', name=f'vaug{m}')
                            for m in mods}
                    qpT = {m: cp_.tile([128, NHT, BS], dt.bfloat16,
                                       tag=f'qpT{m}', name=f'qpT{m}')
                           for m in mods}
                    for m in mods:
                        # only the per-segment ones column (index HD) needs
                        # init: data columns are written by the V-proj drain
                        nc.vector.memset(
                            vaug[m][:].rearrange("p st (h s) -> p st h s",
                                                 h=H)[:, :, :, HD], 1.0)

                    b1x = {}
                    for M in ['img', 'txt']:
                        b1x[M] = spC.tile([128, NT_HID], dt.float32,
                                          tag=f'b1x{M}', name=f'b1x{M}')
                        nc.sync.dma_start(
                            b1x[M][:],
                            inp[f'b1_{M}'].ap().rearrange("(t p) -> p t", p=128))

                    def kv_chunk(m, j):
                        # j in [0, 2*SC): first SC chunks: kp eo-groups;
                        # rest: vaug 4-st groups
                        if j < SC:
                            eo = j
                            for sc in range(SC):
                                ps = psB.tile([128, 512], dt.float32, tag='ps_kv')
                                for k in range(KT_E):
                                    nc.tensor.matmul(
                                        ps[:], wk[m][:, k, eo * 128:(eo + 1) * 128],
                                        rmT[m][:, 2 * sc:2 * sc + 2, k, :],
                                        start=(k == 0), stop=(k == KT_E - 1))
                                nc.vector.tensor_copy(
                                    kpT[m][:, eo, sc * 512:(sc + 1) * 512], ps[:])
                        else:
                            for st in range((j - SC) * 4, (j - SC) * 4 + 4):
                                ps = psB.tile([128, E], dt.float32, tag='ps_kv')
                                for k in range(KT_E):
                                    nc.tensor.matmul(
                                        ps[:],
                                        rmT[m][:, st // 2, k,
                                               (st % 2) * 128:(st % 2) * 128 + 128],
                                        wv[m][:, k, :],
                                        start=(k == 0), stop=(k == KT_E - 1))
                                nc.vector.tensor_tensor(
                                    out=vaug[m][:, st, :].rearrange(
                                        "p (h s) -> p h s", h=H)[:, :, 0:HD],
                                    in0=ps[:].rearrange("p (h d) -> p h d", h=H),
                                    in1=bvb[m][:].rearrange("p (h d) -> p h d", h=H),
                                    op=mybir.AluOpType.add)

                    def qproj(m):
                        for eo in range(NHT):
                            psf = psS.tile([128, 2 * BS], dt.float32,
                                           tag='ps_s', name='ps_qf')
                            ps = psf[:, 0:BS]
                            for k in range(KT_E):
                                nc.tensor.matmul(
                                    ps[:], wq[m][:, k, eo * 128:(eo + 1) * 128],
                                    xTbf[m][:, k, :], start=(k == 0),
                                    stop=(k == KT_E - 1))
                            nc.vector.tensor_scalar_add(qpT[m][:, eo, :], ps[:],
                                                        bqc[m][:, eo:eo + 1])

                    # W1*x precompute chunks (PE bubble filler; DRAM staging)
                    h1x_jobs = [(M, ht) for M in ['img', 'txt']
                                for ht in range(NT_HID)]
                    h1x_pos = [0]

                    def h1x_chunk(n=1):
                        for _ in range(n):
                            if h1x_pos[0] >= len(h1x_jobs):
                                return
                            M, ht = h1x_jobs[h1x_pos[0]]
                            h1x_pos[0] += 1
                            wblk = w1p.tile([128, KT_E, 128], dt.bfloat16,
                                            tag='w1xblk')
                            nc.sync.dma_start(
                                wblk[:],
                                inp[f'w1T_{M}'].ap()[ht, 0:KT_E]
                                .rearrange("k p c -> p k c"))
                            ps = psB.tile([128, B2], dt.float32, tag='ps_kv')
                            for k in range(KT_E):
                                nc.tensor.matmul(ps[:], wblk[:, k, :],
                                                 inT[:, k, :],
                                                 start=(k == 0),
                                                 stop=(k == KT_E - 1))
                            hx = w1p.tile([128, B2], dt.bfloat16, tag='h1x_sb')
                            nc.vector.tensor_scalar_add(hx[:], ps[:],
                                                        b1x[M][:, ht:ht + 1])
                            nc.sync.dma_start(h1x_dram[M].ap()[ht], hx[:])

                    def mha(m, mi, filler):
                        pso = [psO.tile([SEG, HPT * BS], dt.float32,
                                        tag=f'pso{g}', name=f'pso{g}')
                               for g in range(H // HPT)]
                        for st2 in range(0, ST, 2):
                            ex = ep.tile([128, H, 2 * BS], dt.bfloat16,
                                         tag='expT')
                            for g in range(H // HPT):
                                for hh in range(HPT):
                                    h = g * HPT + hh
                                    hb = hh * HD
                                    pss = psS.tile([128, 2 * BS], dt.float32,
                                                   tag='ps_s')
                                    for sj in range(2):
                                        st = st2 + sj
                                        nc.tensor.matmul(
                                            pss[:, sj * BS:(sj + 1) * BS],
                                            kpT[m][hb:hb + HD, g,
                                                   st * 128:(st + 1) * 128],
                                            qpT[m][hb:hb + HD, g, :],
                                            start=True, stop=True,
                                            skip_group_check=True)
                                    nc.scalar.activation(
                                        ex[:, h, :], pss[:], AF.Exp,
                                        bias=0.0,
                                        scale=float(1.0 / np.sqrt(HD)))
                                    for sj in range(2):
                                        st = st2 + sj
                                        nc.tensor.matmul(
                                            pso[g][:, hh * BS:(hh + 1) * BS],
                                            vaug[m][:, st, h * SEG:(h + 1) * SEG],
                                            ex[:, h, sj * BS:(sj + 1) * BS],
                                            start=(st == 0), stop=(st == ST - 1),
                                            skip_group_check=True)
                            filler(st2)
                        # z-row extract (Scalar), broadcast (GpSimd), then a
                        # wide reciprocal (partition-parallel, fast on DVE)
                        zr = spZ.tile([1, H * BS], dt.float32, tag='zr')
                        for h in range(H):
                            nc.scalar.copy(
                                zr[0:1, h * BS:(h + 1) * BS],
                                pso[h // HPT][HD:HD + 1,
                                              (h % HPT) * BS:(h % HPT + 1) * BS])
                        zb = spZ.tile([HD, H * BS], dt.float32, tag='zb')
                        nc.gpsimd.partition_broadcast(zb[:], zr[:])
                        zbi = spZ.tile([HD, H * BS], dt.float32, tag='zbi')
                        nc.vector.reciprocal(zbi[:], zb[:])
                        aoT = cp_.tile([128, NHT, BS], dt.bfloat16, tag='aoT',
                                       name=f'aoT{m}')
                        for h in range(H):
                            nc.vector.tensor_tensor(
                                out=aoT[(h % HPT) * HD:(h % HPT + 1) * HD,
                                        h // HPT, :],
                                in0=pso[h // HPT][0:HD,
                                                  (h % HPT) * BS:(h % HPT + 1) * BS],
                                in1=zbi[:, h * BS:(h + 1) * BS],
                                op=mybir.AluOpType.mult)
                        # out projection -> inT enh rows
                        for eo in range(NHT):
                            psf = psS.tile([128, 2 * BS], dt.float32,
                                           tag='ps_s', name='ps_of')
                            ps = psf[:, 0:BS]
                            for k in range(KT_E):
                                nc.tensor.matmul(
                                    ps[:], wo[m][:, k, eo * 128:(eo + 1) * 128],
                                    aoT[:, k, :], start=(k == 0),
                                    stop=(k == KT_E - 1))
                            nc.vector.tensor_scalar_add(
                                inT[:, KT_E + eo, mi * BS:(mi + 1) * BS], ps[:],
                                boc[m][:, eo:eo + 1])

                    # ---- phase C schedule ----
                    for j in range(2 * SC):
                        kv_chunk('i', j)
                    qproj('i')
                    if F_FILL:
                        # first fillers are h1x (no cross-core dependency) in
                        # case the idx-AllGather for 't' is slow; then the
                        # 't' K/V projection chunks
                        kv_t = list(range(2 * SC))

                        def fill_i(st2):
                            if st2 < 4:
                                h1x_chunk(1)
                            elif kv_t:
                                kv_chunk('t', kv_t.pop(0))
                        mha('i', 0, fill_i)
                        while kv_t:
                            kv_chunk('t', kv_t.pop(0))
                        qproj('t')
                        mha('t', 1, lambda st2: h1x_chunk(2))
                    else:
                        mha('i', 0, lambda st2: None)
                        for j in range(2 * SC):
                            kv_chunk('t', j)
                        qproj('t')
                        mha('t', 1, lambda st2: None)
                    h1x_chunk(len(h1x_jobs))

            # ======== Phase D: the four MLPs (two weight passes) ========
            with (
                tc.tile_pool(name="phD", bufs=1) as dp_,
                tc.tile_pool(name="w1s", bufs=3) as wp,
                tc.tile_pool(name="w2s", bufs=3) as w2p,
                tc.tile_pool(name="smD", bufs=2) as spD,
                tc.tile_pool(name="psD", bufs=4, space="PSUM") as psD,
                tc.tile_pool(name="psW", bufs=2, space="PSUM") as psW,
                tc.tile_pool(name="psT", bufs=2, space="PSUM") as psT,
            ):
                h1T = dp_.tile([128, NT_HID, B2], dt.bfloat16, tag='h1T')
                h2T = dp_.tile([128, NT_HID, B2], dt.bfloat16, tag='h2T')
                out_map = {'img': ['image_hash', 'distill_i'],
                           'txt': ['distill_t', 'text_hash']}
                for M in ['img', 'txt']:
                    b2 = spD.tile([128, NT_HID], dt.float32, tag='b2')
                    bc = spD.tile([1, BIT], dt.bfloat16, tag='bc')
                    nc.sync.dma_start(
                        b2[:], inp[f'b2_{M}'].ap().rearrange("(t p) -> p t", p=128))
                    nc.sync.dma_start(bc[:], inp[f'bc_{M}'].ap())

                    b1f = spD.tile([128, NT_HID], dt.float32, tag='b1f')
                    for ht in range(NT_HID):
                        wblk = wp.tile([128, KT_E, 128], dt.bfloat16, tag='w1blk')
                        nc.sync.dma_start(
                            wblk[:],
                            inp[f'w1T_{M}'].ap()[ht, KT_E:KT_E2]
                            .rearrange("k p c -> p k c"))
                        hx = wp.tile([128, B2], dt.bfloat16, tag='h1x_ld')
                        nc.sync.dma_start(hx[:], h1x_dram[M].ap()[ht])
                        ps = psD.tile([128, B2], dt.float32, tag='ps_h12')
                        for k in range(KT_E):
                            nc.tensor.matmul(ps[:], wblk[:, k, :],
                                             inT[:, KT_E + k, :],
                                             start=(k == 0), stop=(k == KT_E - 1))
                        hpre = wp.tile([128, B2], dt.float32, tag='h1pre')
                        nc.vector.tensor_tensor(out=hpre[:], in0=ps[:], in1=hx[:],
                                                op=mybir.AluOpType.add)
                        nc.vector.tensor_scalar_max(h1T[:, ht, :], hpre[:], 0.0)

                    for ht in range(NT_HID):
                        wblk = w2p.tile([128, NT_HID, 128], dt.bfloat16,
                                        tag='w2blk')
                        nc.sync.dma_start(
                            wblk[:],
                            inp[f'w2T_{M}'].ap()[ht].rearrange("k p c -> p k c"))
                        ps = psD.tile([128, B2], dt.float32, tag='ps_h12')
                        for k in range(NT_HID):
                            nc.tensor.matmul(ps[:], wblk[:, k, :], h1T[:, k, :],
                                             start=(k == 0), stop=(k == NT_HID - 1))
                        nc.vector.tensor_scalar(
                            h2T[:, ht, :], ps[:], b2[:, ht:ht + 1], 0.0,
                            op0=mybir.AluOpType.add, op1=mybir.AluOpType.max)

                    # Wc transposed: weights stationary, batch moving (N=512)
                    wc = dp_.tile([128, NT_HID, BIT], dt.bfloat16, tag='wc')
                    nc.sync.dma_start(
                        wc[:], inp[f'wcT_{M}'].ap().rearrange("k p c -> p k c"))
                    psw = psW.tile([BIT, B2], dt.float32, tag='ps_wc')
                    for k in range(NT_HID):
                        nc.tensor.matmul(psw[:], wc[:, k, :], h2T[:, k, :],
                                         start=(k == 0), stop=False)
                    nc.tensor.matmul(psw[:], bc[:], ones512[:],
                                     start=False, stop=True)
                    h3T = spD.tile([BIT, B2], dt.float32, tag='h3T')
                    nc.scalar.copy(h3T[:], psw[:])
                    for j in range(B2 // 128):
                        pst = psT.tile([128, BIT], dt.float32, tag='ps_t')
                        nc.tensor.transpose(pst[:], h3T[:, j * 128:(j + 1) * 128],
                                            ident[0:BIT, 0:BIT])
                        sq = spD.tile([128, BIT], dt.float32, tag='sq')
                        ss = spD.tile([128, 1], dt.float32, tag='ss')
                        nc.scalar.activation(sq[:], pst[:], AF.Square,
                                             accum_out=ss[:])
                        rs = spD.tile([128, 1], dt.float32, tag='rs')
                        nc.vector.reciprocal(rs[:], ss[:])
                        rsq = spD.tile([128, 1], dt.float32, tag='rsq')
                        nc.scalar.sqrt(rsq[:], rs[:])
                        h3 = spD.tile([128, BIT], dt.float32, tag='h3')
                        nc.vector.tensor_scalar_mul(h3[:], pst[:], rsq[:])
                        oname = out_map[M][j // LT]
                        row = (j % LT) * 128
                        nc.sync.dma_start(outs[oname].ap()[row:row + 128, :],
                                          h3[:])

    nc.compile()
    return nc


def _prep_in_maps(cfg, n_cores, image_feature, text_feature, prompts,
                  img_in_w, img_in_b, img_out_w, img_out_b,
                  txt_in_w, txt_in_b, txt_out_w, txt_out_b,
                  img_W1, img_b1, img_W2, img_b2, img_Wc, img_bc,
                  txt_W1, txt_b1, txt_W2, txt_b2, txt_Wc, txt_bc):
    C = cfg
    E, P, BIT, BS = C['E'], C['P'], C['BIT'], C['BS']
    NT_HID, KT_E2 = C['NT_HID'], C['KT_E2']

    def bt(x):
        return np.ascontiguousarray(np.asarray(x).astype(BF16))

    common = {}
    common['promptsT'] = np.ascontiguousarray(prompts.T.astype(np.float32))
    common['prompts_bf'] = bt(prompts)

    for m, in_w, in_b, out_w, out_b in [
            ('i', img_in_w, img_in_b, img_out_w, img_out_b),
            ('t', txt_in_w, txt_in_b, txt_out_w, txt_out_b)]:
        common[f'wqT_{m}'] = bt(in_w[:E].T)
        common[f'wkT_{m}'] = bt(in_w[E:2 * E].T)
        common[f'wvT_{m}'] = bt(in_w[2 * E:].T)
        common[f'woT_{m}'] = bt(out_w.T)
        common[f'bq_{m}'] = np.ascontiguousarray(in_b[:E].astype(np.float32))
        common[f'bv_{m}'] = bt(in_b[2 * E:][None, :])
        common[f'bo_{m}'] = np.ascontiguousarray(out_b.astype(np.float32))

    for M, W1, b1, W2, b2, Wc, bc in [
            ('img', img_W1, img_b1, img_W2, img_b2, img_Wc, img_bc),
            ('txt', txt_W1, txt_b1, txt_W2, txt_b2, txt_Wc, txt_bc)]:
        w1t = np.asarray(W1).T.astype(BF16)      # [2E, HID]
        common[f'w1T_{M}'] = np.ascontiguousarray(
            w1t.reshape(KT_E2, 128, NT_HID, 128).transpose(2, 0, 1, 3))
        w2t = np.asarray(W2).T.astype(BF16)      # [HID, HID]
        common[f'w2T_{M}'] = np.ascontiguousarray(
            w2t.reshape(NT_HID, 128, NT_HID, 128).transpose(2, 0, 1, 3))
        wct = np.asarray(Wc).T.astype(BF16)      # [HID, BIT]
        common[f'wcT_{M}'] = np.ascontiguousarray(wct.reshape(NT_HID, 128, BIT))
        common[f'b1_{M}'] = np.ascontiguousarray(b1.astype(np.float32))
        common[f'b2_{M}'] = np.ascontiguousarray(b2.astype(np.float32))
        common[f'bc_{M}'] = bt(np.asarray(bc)[None, :])

    xTi = np.asarray(image_feature).T.astype(np.float32)
    xTt = np.asarray(text_feature).T.astype(np.float32)
    in_maps = []
    for c in range(n_cores):
        im = dict(common)
        im['xT_i'] = np.ascontiguousarray(xTi[:, c * BS:(c + 1) * BS])
        im['xT_t'] = np.ascontiguousarray(xTt[:, c * BS:(c + 1) * BS])
        in_maps.append(im)
    return in_maps


_NC_CACHE = {}


def _get_nc(cfg, n_cores):
    key = (tuple(sorted(cfg.items())), n_cores)
    if key not in _NC_CACHE:
        _NC_CACHE[key] = build_nc(cfg, n_cores)
    return _NC_CACHE[key]


def run(inputs, cfg=None, n_cores=None, trace=False):
    cfg = cfg or _cfg(**FULL)
    n_cores = n_cores or cfg['NC']
    nc = _get_nc(cfg, n_cores)
    in_maps = _prep_in_maps(cfg, n_cores, **{
        k: np.asarray(v) for k, v in inputs.items() if k != 'iteration'})
    res = run_bass_kernel_spmd(nc, in_maps, list(range(n_cores)), trace=trace)
    out = {}
    for name in ['image_hash', 'text_hash', 'distill_i', 'distill_t']:
        out[name] = np.concatenate(
            [res.results[c][name] for c in range(n_cores)], axis=0)
    return (out['image_hash'], out['text_hash'],
            out['distill_i'], out['distill_t']), res


def kernel(**inputs):
    (ih, th, di, dtl), _ = run(inputs)
    return ih, th, di, dtl


# revision 28
# speedup vs baseline: 1.0466x; 1.0030x over previous
"""Trainium2 Bass kernel for nn_HashingModel (retrieval_knn).

Sharding: data-parallel over batch B across 8 cores (256 rows each).

v2 design (vs v1's K/V AllGather):
- Cross-batch MHA needs K/V for all 2048 keys. Every core already holds the
  full prompt table in DRAM, so we AllGather only the argmax *indices*
  (1 KB vs 512 KB), gather all 2048 prompt rows locally, and project K/V
  for the full batch on every core. The extra ~30us of replicated matmul
  per modality replaces 47-84us AllGather stalls that also tripped the
  PE activity throttle (HAM K=4/8) for whole phases.
- Similarity+argmax in fp32 (argmax must match the reference exactly;
  fp32 matmul measured at 2 cyc/row on HW). Sim drains on the Scalar
  engine so the Vector engine is free for argmax.
- K-proj bias dropped (softmax row-shift invariant; zero in practice),
  V-proj bias folded into the value table drain.
- MHA score/exp/AV loop is exp(Scalar)-paced; PE bubbles are filled by
  interleaving the other modality's K/V projection and W1*x precompute.
- Final Wc layer runs transposed (weights stationary, batch moving,
  N=512 instead of N=64) + PE transpose before the l2norm chain.
- All heavy matmuls bf16; weights pre-transposed host-side.

Self-contained: hardcoded shapes, no file reads.
"""
import sys
import numpy as np

sys.path.insert(0, '/opt/trn_rl_repo')

import ml_dtypes
from concourse import bass, bacc, tile, mybir
from concourse.bass_utils import run_bass_kernel_spmd
from concourse.masks import make_identity

dt = mybir.dt
BF16 = ml_dtypes.bfloat16
AF = mybir.ActivationFunctionType

FULL = dict(NC=8, B=2048, E=512, P=4096, H=8, HD=64, HID=4096, BIT=64)


def _cfg(NC, B, E, P, H, HD, HID, BIT):
    c = dict(NC=NC, B=B, E=E, P=P, H=H, HD=HD, HID=HID, BIT=BIT)
    c['BS'] = B // NC          # batch shard per core
    c['E2'] = 2 * E            # MLP input dim
    c['KT_E'] = E // 128       # k-tiles over E
    c['KT_E2'] = 2 * E // 128
    c['NT_HID'] = HID // 128
    c['LT'] = c['BS'] // 128   # l-tiles per shard
    c['ST'] = B // 128         # s-tiles over full batch
    c['PC'] = P // 512         # prompt chunks for sim
    c['B2'] = 2 * c['BS']      # MLP free dim (fi|ft)
    return c


def build_nc(cfg, n_cores):
    import os
    F_FILL = os.environ.get('F_FILL', '1') == '1'
    C = cfg
    NC = n_cores
    E, P, H, HD, HID, BIT = C['E'], C['P'], C['H'], C['HD'], C['HID'], C['BIT']
    BS, E2 = C['BS'], C['E2']
    KT_E, KT_E2, NT_HID, LT, ST, PC, B2 = (C['KT_E'], C['KT_E2'], C['NT_HID'],
                                           C['LT'], C['ST'], C['PC'], C['B2'])
    HPT = 128 // HD            # heads per 128-partition tile (2)
    NHT = E // 128             # eo tiles (4)
    SEG = HD + 1               # vaug segment width (65)
    S = C['B']                 # full batch (attention keys)
    SC = S // 512              # 512-col chunks of S
    FR = S // 16               # wrap16 columns for gather indices

    nc = bacc.Bacc("TRN2", target_bir_lowering=False, debug=False,
                   num_devices=NC)

    mods = ['i', 't']
    inp = {}

    def din(name, shape, d):
        inp[name] = nc.dram_tensor(name, shape, d, kind="ExternalInput")

    # weight layouts put each SBUF partition's data contiguous in DRAM so
    # every weight-block DMA is one large descriptor per partition
    for m in mods:
        din(f'xT_{m}', [E, BS], dt.float32)
        for w in ['wqT', 'wkT', 'wvT', 'woT']:
            din(f'{w}_{m}', [128, KT_E, E], dt.bfloat16)
        din(f'bq_{m}', [E], dt.float32)
        din(f'bo_{m}', [E], dt.float32)
        din(f'bv_{m}', [1, E], dt.bfloat16)
    din('promptsT', [E, P], dt.float32)
    din('prompts_bf', [P, E], dt.bfloat16)
    for M in ['img', 'txt']:
        din(f'w1T_{M}', [NT_HID, 128, KT_E2, 128], dt.bfloat16)
        din(f'w2T_{M}', [NT_HID, 128, NT_HID, 128], dt.bfloat16)
        din(f'wcT_{M}', [128, NT_HID, BIT], dt.bfloat16)
        din(f'b1_{M}', [HID], dt.float32)
        din(f'b2_{M}', [HID], dt.float32)
        din(f'bc_{M}', [1, BIT], dt.bfloat16)

    outs = {}
    for name in ['image_hash', 'text_hash', 'distill_i', 'distill_t']:
        outs[name] = nc.dram_tensor(name, [BS, BIT], dt.float32,
                                    kind="ExternalOutput")

    idx_in = {m: nc.dram_tensor(f'idx_in_{m}', [BS], dt.uint32) for m in mods}
    idx_out = {m: nc.dram_tensor(f'idx_out_{m}', [NC * BS], dt.uint32,
                                 addr_space="Shared") for m in mods}
    h1x_dram = {M: nc.dram_tensor(f'h1x_{M}', [NT_HID, 128, B2], dt.bfloat16)
                for M in ['img', 'txt']}

    with tile.TileContext(nc) as tc:
        with tc.tile_pool(name="persist", bufs=1) as pp:
            xTbf = {m: pp.tile([128, KT_E, BS], dt.bfloat16, tag=f'xTbf{m}',
                               name=f'xTbf{m}') for m in mods}
            inT = pp.tile([128, KT_E2, B2], dt.bfloat16, tag='inT')
            ones512 = pp.tile([1, 512], dt.bfloat16, tag='ones')
            nc.vector.memset(ones512[:], 1.0)
            ident = pp.tile([128, 128], dt.float32, tag='ident')
            make_identity(nc, ident[:])

            # weights + small tensors that live through phases A-C
            with tc.tile_pool(name="wts", bufs=1) as wt:
                wk, wv, wq, wo, bvb, bqc, boc = {}, {}, {}, {}, {}, {}, {}
                for m in mods:
                    wk[m] = wt.tile([128, KT_E, E], dt.bfloat16, tag=f'wk{m}',
                                    name=f'wk{m}')
                    wv[m] = wt.tile([128, KT_E, E], dt.bfloat16, tag=f'wv{m}',
                                    name=f'wv{m}')
                    wq[m] = wt.tile([128, KT_E, E], dt.bfloat16, tag=f'wq{m}',
                                    name=f'wq{m}')
                    wo[m] = wt.tile([128, KT_E, E], dt.bfloat16, tag=f'wo{m}',
                                    name=f'wo{m}')
                    nc.sync.dma_start(wk[m][:], inp[f'wkT_{m}'].ap())
                    nc.sync.dma_start(wv[m][:], inp[f'wvT_{m}'].ap())
                    nc.sync.dma_start(wq[m][:], inp[f'wqT_{m}'].ap())
                    nc.sync.dma_start(wo[m][:], inp[f'woT_{m}'].ap())
                    bvr = wt.tile([1, E], dt.bfloat16, tag=f'bvr{m}',
                                  name=f'bvr{m}')
                    nc.sync.dma_start(bvr[:], inp[f'bv_{m}'].ap())
                    bvb[m] = wt.tile([128, E], dt.bfloat16, tag=f'bvb{m}',
                                     name=f'bvb{m}')
                    nc.gpsimd.partition_broadcast(bvb[m][:], bvr[:])
                    bqc[m] = wt.tile([128, NHT], dt.float32, tag=f'bq{m}',
                                     name=f'bq{m}')
                    boc[m] = wt.tile([128, NHT], dt.float32, tag=f'bo{m}',
                                     name=f'bo{m}')
                    nc.sync.dma_start(
                        bqc[m][:],
                        inp[f'bq_{m}'].ap().rearrange("(t p) -> p t", p=128))
                    nc.sync.dma_start(
                        boc[m][:],
                        inp[f'bo_{m}'].ap().rearrange("(t p) -> p t", p=128))
                # gathered prompt rows, chunked so each 256-idx sub-gather
                # writes a contiguous [128, KT_E, 256] block
                JC = S // 256
                rmT = {m: wt.tile([128, JC, KT_E, 256], dt.bfloat16,
                                  tag=f'rmT{m}', name=f'rmT{m}') for m in mods}

                # ======== Phase A: sim -> argmax -> idx AllGather ->
                # local gather of all 2048 prompt rows per modality ========
                with (
                    tc.tile_pool(name="phA", bufs=1) as ap_,
                    tc.tile_pool(name="simbuf", bufs=2) as simp,
                    tc.tile_pool(name="smA", bufs=2) as sp,
                    tc.tile_pool(name="psA", bufs=4, space="PSUM") as psA,
                ):
                    xT32 = {m: ap_.tile([128, KT_E, BS], dt.float32,
                                        tag=f'xT32{m}', name=f'xT32{m}')
                            for m in mods}
                    for mi, m in enumerate(mods):
                        nc.sync.dma_start(
                            xT32[m][:],
                            inp[f'xT_{m}'].ap().rearrange("(k p) b -> p k b", p=128))
                        nc.vector.tensor_copy(xTbf[m][:], xT32[m][:])
                        nc.vector.tensor_copy(
                            inT[:, 0:KT_E, mi * BS:(mi + 1) * BS], xTbf[m][:])
                    prT = ap_.tile([128, KT_E, P], dt.float32, tag='promptsT')
                    prsrc = inp['promptsT'].ap().rearrange(
                        "(k p) (h n) -> k p h n", p=128, h=2)
                    for hf in range(2):
                        for k in range(KT_E):
                            nc.sync.dma_start(
                                prT[:, k, hf * (P // 2):(hf + 1) * (P // 2)],
                                prsrc[k, :, hf])

                    for m in mods:
                        sims = []
                        for lt in range(LT):
                            sim = simp.tile([128, P], dt.float32, tag='sim')
                            sims.append(sim)
                            for pc in range(PC):
                                ps = psA.tile([128, 512], dt.float32,
                                              tag='ps_sim')
                                for k in range(KT_E):
                                    nc.tensor.matmul(
                                        ps[:],
                                        xT32[m][:, k, lt * 128:(lt + 1) * 128],
                                        prT[:, k, pc * 512:(pc + 1) * 512],
                                        start=(k == 0), stop=(k == KT_E - 1))
                                # drain on Scalar engine: DVE stays free for
                                # argmax, PE for sim
                                nc.scalar.copy(sim[:, pc * 512:(pc + 1) * 512],
                                               ps[:])
                            m8 = sp.tile([128, 8], dt.float32, tag='m8')
                            i8 = sp.tile([128, 8], dt.uint32, tag='i8')
                            nc.vector.max(m8[:], sim[:])
                            nc.vector.max_index(i8[:], m8[:], sim[:])
                            nc.sync.dma_start(
                                idx_in[m].ap()[lt * 128:(lt + 1) * 128],
                                i8[:, 0:1])
                        nc.gpsimd.collective_compute(
                            "AllGather", mybir.AluOpType.bypass,
                            replica_groups=[list(range(NC))],
                            ins=[idx_in[m][:]], outs=[idx_out[m][:]])
                        # wrap16 + replicate + int16 for the gpsimd gather
                        i32 = sp.tile([128, FR], dt.uint32, tag='i32g')
                        for a in range(2):
                            nc.sync.dma_start(
                                i32[16 * a:16 * (a + 1), :],
                                idx_out[m].ap().rearrange("(f p) -> p f", p=16))
                        nc.vector.tensor_copy(i32[32:64, :], i32[0:32, :])
                        nc.vector.tensor_copy(i32[64:128, :], i32[0:64, :])
                        ix16 = sp.tile([128, FR], dt.int16, tag='i16g')
                        nc.vector.tensor_copy(ix16[:], i32[:])
                        for j in range(S // 256):
                            nc.gpsimd.dma_gather(
                                rmT[m][:, j], inp['prompts_bf'].ap(),
                                ix16[:, 16 * j:16 * (j + 1)],
                                num_idxs=256, num_idxs_reg=256, elem_size=E,
                                transpose=True)

                # ======== Phase C: K/V proj (full batch, local) + MHA.
                # The other modality's K/V projection and the W1*x
                # precompute fill the exp-paced PE bubbles. ========
                with (
                    tc.tile_pool(name="phC", bufs=1) as cp_,
                    tc.tile_pool(name="w1xp", bufs=3) as w1p,
                    tc.tile_pool(name="expp", bufs=2) as ep,
                    tc.tile_pool(name="smC", bufs=2) as spC,
                    tc.tile_pool(name="smZ", bufs=1) as spZ,
                    tc.tile_pool(name="psB", bufs=2, space="PSUM") as psB,
                    tc.tile_pool(name="psS", bufs=2, space="PSUM") as psS,
                    tc.tile_pool(name="psO", bufs=1, space="PSUM") as psO,
                ):
                    kpT = {m: cp_.tile([128, NHT, S], dt.bfloat16,
                                       tag=f'kpT{m}', name=f'kpT{m}')
                           for m in mods}
                    vaug = {m: cp_.tile([128, ST, H * SEG], dt.bfloat16,
                                        tag=f'vaug{m}', name=f'vaug{m}')
                            for m in mods}
                    qpT = {m: cp_.tile([128, NHT, BS], dt.bfloat16,
                                       tag=f'qpT{m}', name=f'qpT{m}')
                           for m in mods}
                    for m in mods:
                        # only the per-segment ones column (index HD) needs
                        # init: data columns are written by the V-proj drain
                        nc.vector.memset(
                            vaug[m][:].rearrange("p st (h s) -> p st h s",
                                                 h=H)[:, :, :, HD], 1.0)

                    b1x = {}
                    for M in ['img', 'txt']:
                        b1x[M] = spC.tile([128, NT_HID], dt.float32,
                                          tag=f'b1x{M}', name=f'b1x{M}')
                        nc.sync.dma_start(
                            b1x[M][:],
                            inp[f'b1_{M}'].ap().rearrange("(t p) -> p t", p=128))

                    def kv_chunk(m, j):
                        # j in [0, 2*SC): first SC chunks: kp eo-groups;
                        # rest: vaug 4-st groups
                        if j < SC:
                            eo = j
                            for sc in range(SC):
                                ps = psB.tile([128, 512], dt.float32, tag='ps_kv')
                                for k in range(KT_E):
                                    nc.tensor.matmul(
                                        ps[:], wk[m][:, k, eo * 128:(eo + 1) * 128],
                                        rmT[m][:, 2 * sc:2 * sc + 2, k, :],
                                        start=(k == 0), stop=(k == KT_E - 1))
                                nc.vector.tensor_copy(
                                    kpT[m][:, eo, sc * 512:(sc + 1) * 512], ps[:])
                        else:
                            for st in range((j - SC) * 4, (j - SC) * 4 + 4):
                                ps = psB.tile([128, E], dt.float32, tag='ps_kv')
                                for k in range(KT_E):
                                    nc.tensor.matmul(
                                        ps[:],
                                        rmT[m][:, st // 2, k,
                                               (st % 2) * 128:(st % 2) * 128 + 128],
                                        wv[m][:, k, :],
                                        start=(k == 0), stop=(k == KT_E - 1))
                                nc.vector.tensor_tensor(
                                    out=vaug[m][:, st, :].rearrange(
                                        "p (h s) -> p h s", h=H)[:, :, 0:HD],
                                    in0=ps[:].rearrange("p (h d) -> p h d", h=H),
                                    in1=bvb[m][:].rearrange("p (h d) -> p h d", h=H),
                                    op=mybir.AluOpType.add)

                    def qproj(m):
                        for eo in range(NHT):
                            psf = psS.tile([128, 2 * BS], dt.float32,
                                           tag='ps_s', name='ps_qf')
                            ps = psf[:, 0:BS]
                            for k in range(KT_E):
                                nc.tensor.matmul(
                                    ps[:], wq[m][:, k, eo * 128:(eo + 1) * 128],
                                    xTbf[m][:, k, :], start=(k == 0),
                                    stop=(k == KT_E - 1))
                            nc.vector.tensor_scalar_add(qpT[m][:, eo, :], ps[:],
                                                        bqc[m][:, eo:eo + 1])

                    # W1*x precompute chunks (PE bubble filler; DRAM staging).
                    # Only chunks actually consumed as fillers are
                    # precomputed; the rest run full-K in phase D.
                    h1x_jobs = [(M, ht) for M in ['img', 'txt']
                                for ht in range(NT_HID)]
                    h1x_pos = [0]
                    h1x_done = set()

                    def h1x_chunk(n=1):
                        for _ in range(n):
                            if h1x_pos[0] >= len(h1x_jobs):
                                return
                            M, ht = h1x_jobs[h1x_pos[0]]
                            h1x_pos[0] += 1
                            h1x_done.add((M, ht))
                            wblk = w1p.tile([128, KT_E, 128], dt.bfloat16,
                                            tag='w1xblk')
                            nc.sync.dma_start(
                                wblk[:], inp[f'w1T_{M}'].ap()[ht, :, 0:KT_E])
                            ps = psB.tile([128, B2], dt.float32, tag='ps_kv')
                            for k in range(KT_E):
                                nc.tensor.matmul(ps[:], wblk[:, k, :],
                                                 inT[:, k, :],
                                                 start=(k == 0),
                                                 stop=(k == KT_E - 1))
                            hx = w1p.tile([128, B2], dt.bfloat16, tag='h1x_sb')
                            nc.vector.tensor_scalar_add(hx[:], ps[:],
                                                        b1x[M][:, ht:ht + 1])
                            nc.sync.dma_start(h1x_dram[M].ap()[ht], hx[:])

                    def mha_loop(m, filler):
                        pso = [psO.tile([SEG, HPT * BS], dt.float32,
                                        tag=f'pso{g}', name=f'pso{g}')
                               for g in range(H // HPT)]
                        for st2 in range(0, ST, 2):
                            ex = ep.tile([128, H, 2 * BS], dt.bfloat16,
                                         tag='expT')
                            for g in range(H // HPT):
                                for hh in range(HPT):
                                    h = g * HPT + hh
                                    hb = hh * HD
                                    pss = psS.tile([128, 2 * BS], dt.float32,
                                                   tag='ps_s')
                                    for sj in range(2):
                                        st = st2 + sj
                                        nc.tensor.matmul(
                                            pss[:, sj * BS:(sj + 1) * BS],
                                            kpT[m][hb:hb + HD, g,
                                                   st * 128:(st + 1) * 128],
                                            qpT[m][hb:hb + HD, g, :],
                                            start=True, stop=True,
                                            skip_group_check=True)
                                    nc.scalar.activation(
                                        ex[:, h, :], pss[:], AF.Exp,
                                        bias=0.0,
                                        scale=float(1.0 / np.sqrt(HD)))
                                    for sj in range(2):
                                        st = st2 + sj
                                        nc.tensor.matmul(
                                            pso[g][:, hh * BS:(hh + 1) * BS],
                                            vaug[m][:, st, h * SEG:(h + 1) * SEG],
                                            ex[:, h, sj * BS:(sj + 1) * BS],
                                            start=(st == 0), stop=(st == ST - 1),
                                            skip_group_check=True)
                            filler(st2)
                        return pso

                    def mha_finish(m, mi, pso):
                        # z-row extract (Scalar), broadcast (GpSimd), then a
                        # wide reciprocal (partition-parallel, fast on DVE)
                        zr = spZ.tile([1, H * BS], dt.float32, tag='zr')
                        for h in range(H):
                            nc.scalar.copy(
                                zr[0:1, h * BS:(h + 1) * BS],
                                pso[h // HPT][HD:HD + 1,
                                              (h % HPT) * BS:(h % HPT + 1) * BS])
                        zb = spZ.tile([HD, H * BS], dt.float32, tag='zb')
                        nc.gpsimd.partition_broadcast(zb[:], zr[:])
                        zbi = spZ.tile([HD, H * BS], dt.float32, tag='zbi')
                        nc.vector.reciprocal(zbi[:], zb[:])
                        aoT = cp_.tile([128, NHT, BS], dt.bfloat16, tag='aoT',
                                       name=f'aoT{m}')
                        for h in range(H):
                            nc.vector.tensor_tensor(
                                out=aoT[(h % HPT) * HD:(h % HPT + 1) * HD,
                                        h // HPT, :],
                                in0=pso[h // HPT][0:HD,
                                                  (h % HPT) * BS:(h % HPT + 1) * BS],
                                in1=zbi[:, h * BS:(h + 1) * BS],
                                op=mybir.AluOpType.mult)
                        # out projection -> inT enh rows
                        for eo in range(NHT):
                            psf = psS.tile([128, 2 * BS], dt.float32,
                                           tag='ps_s', name='ps_of')
                            ps = psf[:, 0:BS]
                            for k in range(KT_E):
                                nc.tensor.matmul(
                                    ps[:], wo[m][:, k, eo * 128:(eo + 1) * 128],
                                    aoT[:, k, :], start=(k == 0),
                                    stop=(k == KT_E - 1))
                            nc.vector.tensor_scalar_add(
                                inT[:, KT_E + eo, mi * BS:(mi + 1) * BS], ps[:],
                                boc[m][:, eo:eo + 1])

                    # ---- phase C schedule ----
                    # h1x chunks before kvproj_i cover the idx-AllGather +
                    # gather latency; the 't' K/V projection runs between the
                    # two attention loops (covering modality-i's z-chain);
                    # out-projections are deferred so the PE never waits on
                    # the Vector/Scalar z work.
                    if F_FILL:
                        h1x_chunk(12)
                    for j in range(2 * SC):
                        kv_chunk('i', j)
                    qproj('i')
                    if F_FILL:
                        pso_i = mha_loop('i', lambda st2: h1x_chunk(2))
                        for j in range(2 * SC):
                            kv_chunk('t', j)
                        qproj('t')
                        mha_finish('i', 0, pso_i)
                        pso_t = mha_loop('t', lambda st2: h1x_chunk(2))
                        mha_finish('t', 1, pso_t)
                    else:
                        pso_i = mha_loop('i', lambda st2: None)
                        mha_finish('i', 0, pso_i)
                        for j in range(2 * SC):
                            kv_chunk('t', j)
                        qproj('t')
                        pso_t = mha_loop('t', lambda st2: None)
                        mha_finish('t', 1, pso_t)

            # ======== Phase D: the four MLPs (two weight passes) ========
            with (
                tc.tile_pool(name="phD", bufs=1) as dp_,
                tc.tile_pool(name="w1s", bufs=3) as wp,
                tc.tile_pool(name="w2s", bufs=3) as w2p,
                tc.tile_pool(name="smD", bufs=2) as spD,
                tc.tile_pool(name="psD", bufs=4, space="PSUM") as psD,
                tc.tile_pool(name="psW", bufs=2, space="PSUM") as psW,
                tc.tile_pool(name="psT", bufs=2, space="PSUM") as psT,
            ):
                h1T = dp_.tile([128, NT_HID, B2], dt.bfloat16, tag='h1T')
                h2T = dp_.tile([128, NT_HID, B2], dt.bfloat16, tag='h2T')
                out_map = {'img': ['image_hash', 'distill_i'],
                           'txt': ['distill_t', 'text_hash']}
                for M in ['img', 'txt']:
                    b2 = spD.tile([128, NT_HID], dt.float32, tag='b2')
                    bc = spD.tile([1, BIT], dt.bfloat16, tag='bc')
                    nc.sync.dma_start(
                        b2[:], inp[f'b2_{M}'].ap().rearrange("(t p) -> p t", p=128))
                    nc.sync.dma_start(bc[:], inp[f'bc_{M}'].ap())

                    b1f = spD.tile([128, NT_HID], dt.float32, tag='b1f')
                    nc.sync.dma_start(
                        b1f[:], inp[f'b1_{M}'].ap().rearrange("(t p) -> p t", p=128))
                    for ht in range(NT_HID):
                        if (M, ht) in h1x_done:
                            wblk = wp.tile([128, KT_E, 128], dt.bfloat16,
                                           tag='w1blk')
                            nc.sync.dma_start(
                                wblk[:], inp[f'w1T_{M}'].ap()[ht, :, KT_E:KT_E2])
                            hx = wp.tile([128, B2], dt.bfloat16, tag='h1x_ld')
                            nc.sync.dma_start(hx[:], h1x_dram[M].ap()[ht])
                            ps = psD.tile([128, B2], dt.float32, tag='ps_h12')
                            for k in range(KT_E):
                                nc.tensor.matmul(ps[:], wblk[:, k, :],
                                                 inT[:, KT_E + k, :],
                                                 start=(k == 0),
                                                 stop=(k == KT_E - 1))
                            hpre = wp.tile([128, B2], dt.float32, tag='h1pre')
                            nc.vector.tensor_tensor(out=hpre[:], in0=ps[:],
                                                    in1=hx[:],
                                                    op=mybir.AluOpType.add)
                            nc.vector.tensor_scalar_max(h1T[:, ht, :], hpre[:],
                                                        0.0)
                        else:
                            wblk = wp.tile([128, KT_E2, 128], dt.bfloat16,
                                           tag='w1blkf', name='w1blkf')
                            nc.sync.dma_start(
                                wblk[:], inp[f'w1T_{M}'].ap()[ht])
                            ps = psD.tile([128, B2], dt.float32, tag='ps_h12')
                            for k in range(KT_E2):
                                nc.tensor.matmul(ps[:], wblk[:, k, :],
                                                 inT[:, k, :],
                                                 start=(k == 0),
                                                 stop=(k == KT_E2 - 1))
                            nc.vector.tensor_scalar(
                                h1T[:, ht, :], ps[:], b1f[:, ht:ht + 1], 0.0,
                                op0=mybir.AluOpType.add, op1=mybir.AluOpType.max)

                    for ht in range(NT_HID):
                        wblk = w2p.tile([128, NT_HID, 128], dt.bfloat16,
                                        tag='w2blk')
                        nc.sync.dma_start(
                            wblk[:], inp[f'w2T_{M}'].ap()[ht])
                        ps = psD.tile([128, B2], dt.float32, tag='ps_h12')
                        for k in range(NT_HID):
                            nc.tensor.matmul(ps[:], wblk[:, k, :], h1T[:, k, :],
                                             start=(k == 0), stop=(k == NT_HID - 1))
                        nc.vector.tensor_scalar(
                            h2T[:, ht, :], ps[:], b2[:, ht:ht + 1], 0.0,
                            op0=mybir.AluOpType.add, op1=mybir.AluOpType.max)

                    # Wc transposed: weights stationary, batch moving (N=512)
                    wc = dp_.tile([128, NT_HID, BIT], dt.bfloat16, tag='wc')
                    nc.sync.dma_start(wc[:], inp[f'wcT_{M}'].ap())
                    psw = psW.tile([BIT, B2], dt.float32, tag='ps_wc')
                    for k in range(NT_HID):
                        nc.tensor.matmul(psw[:], wc[:, k, :], h2T[:, k, :],
                                         start=(k == 0), stop=False)
                    nc.tensor.matmul(psw[:], bc[:], ones512[:],
                                     start=False, stop=True)
                    h3T = spD.tile([BIT, B2], dt.float32, tag='h3T')
                    nc.scalar.copy(h3T[:], psw[:])
                    for j in range(B2 // 128):
                        pst = psT.tile([128, BIT], dt.float32, tag='ps_t')
                        nc.tensor.transpose(pst[:], h3T[:, j * 128:(j + 1) * 128],
                                            ident[0:BIT, 0:BIT])
                        sq = spD.tile([128, BIT], dt.float32, tag='sq')
                        ss = spD.tile([128, 1], dt.float32, tag='ss')
                        nc.scalar.activation(sq[:], pst[:], AF.Square,
                                             accum_out=ss[:])
                        rs = spD.tile([128, 1], dt.float32, tag='rs')
                        nc.vector.reciprocal(rs[:], ss[:])
                        rsq = spD.tile([128, 1], dt.float32, tag='rsq')
                        nc.scalar.sqrt(rsq[:], rs[:])
                        h3 = spD.tile([128, BIT], dt.float32, tag='h3')
                        nc.vector.tensor_scalar_mul(h3[:], pst[:], rsq[:])
                        oname = out_map[M][j // LT]
                        row = (j % LT) * 128
                        nc.sync.dma_start(outs[oname].ap()[row:row + 128, :],
                                          h3[:])

    nc.compile()
    return nc


def _prep_in_maps(cfg, n_cores, image_feature, text_feature, prompts,
                  img_in_w, img_in_b, img_out_w, img_out_b,
                  txt_in_w, txt_in_b, txt_out_w, txt_out_b,
                  img_W1, img_b1, img_W2, img_b2, img_Wc, img_bc,
                  txt_W1, txt_b1, txt_W2, txt_b2, txt_Wc, txt_bc):
    C = cfg
    E, P, BIT, BS = C['E'], C['P'], C['BIT'], C['BS']
    NT_HID, KT_E2 = C['NT_HID'], C['KT_E2']

    def bt(x):
        return np.ascontiguousarray(np.asarray(x).astype(BF16))

    common = {}
    common['promptsT'] = np.ascontiguousarray(prompts.T.astype(np.float32))
    common['prompts_bf'] = bt(prompts)

    KT_E = E // 128

    def kpn(w):  # [E, N] -> [128, KT_E, N] (partition-contiguous in DRAM)
        w = np.asarray(w).astype(BF16)
        return np.ascontiguousarray(
            w.reshape(KT_E, 128, w.shape[1]).transpose(1, 0, 2))

    for m, in_w, in_b, out_w, out_b in [
            ('i', img_in_w, img_in_b, img_out_w, img_out_b),
            ('t', txt_in_w, txt_in_b, txt_out_w, txt_out_b)]:
        common[f'wqT_{m}'] = kpn(in_w[:E].T)
        common[f'wkT_{m}'] = kpn(in_w[E:2 * E].T)
        common[f'wvT_{m}'] = kpn(in_w[2 * E:].T)
        common[f'woT_{m}'] = kpn(out_w.T)
        common[f'bq_{m}'] = np.ascontiguousarray(in_b[:E].astype(np.float32))
        common[f'bv_{m}'] = bt(in_b[2 * E:][None, :])
        common[f'bo_{m}'] = np.ascontiguousarray(out_b.astype(np.float32))

    for M, W1, b1, W2, b2, Wc, bc in [
            ('img', img_W1, img_b1, img_W2, img_b2, img_Wc, img_bc),
            ('txt', txt_W1, txt_b1, txt_W2, txt_b2, txt_Wc, txt_bc)]:
        w1t = np.asarray(W1).T.astype(BF16)      # [2E, HID]
        common[f'w1T_{M}'] = np.ascontiguousarray(
            w1t.reshape(KT_E2, 128, NT_HID, 128).transpose(2, 1, 0, 3))
        w2t = np.asarray(W2).T.astype(BF16)      # [HID, HID]
        common[f'w2T_{M}'] = np.ascontiguousarray(
            w2t.reshape(NT_HID, 128, NT_HID, 128).transpose(2, 1, 0, 3))
        wct = np.asarray(Wc).T.astype(BF16)      # [HID, BIT]
        common[f'wcT_{M}'] = np.ascontiguousarray(
            wct.reshape(NT_HID, 128, BIT).transpose(1, 0, 2))
        common[f'b1_{M}'] = np.ascontiguousarray(b1.astype(np.float32))
        common[f'b2_{M}'] = np.ascontiguousarray(b2.astype(np.float32))
        common[f'bc_{M}'] = bt(np.asarray(bc)[None, :])

    xTi = np.asarray(image_feature).T.astype(np.float32)
    xTt = np.asarray(text_feature).T.astype(np.float32)
    in_maps = []
    for c in range(n_cores):
        im = dict(common)
        im['xT_i'] = np.ascontiguousarray(xTi[:, c * BS:(c + 1) * BS])
        im['xT_t'] = np.ascontiguousarray(xTt[:, c * BS:(c + 1) * BS])
        in_maps.append(im)
    return in_maps


_NC_CACHE = {}


def _get_nc(cfg, n_cores):
    key = (tuple(sorted(cfg.items())), n_cores)
    if key not in _NC_CACHE:
        _NC_CACHE[key] = build_nc(cfg, n_cores)
    return _NC_CACHE[key]


def run(inputs, cfg=None, n_cores=None, trace=False):
    cfg = cfg or _cfg(**FULL)
    n_cores = n_cores or cfg['NC']
    nc = _get_nc(cfg, n_cores)
    in_maps = _prep_in_maps(cfg, n_cores, **{
        k: np.asarray(v) for k, v in inputs.items() if k != 'iteration'})
    res = run_bass_kernel_spmd(nc, in_maps, list(range(n_cores)), trace=trace)
    out = {}
    for name in ['image_hash', 'text_hash', 'distill_i', 'distill_t']:
        out[name] = np.concatenate(
            [res.results[c][name] for c in range(n_cores)], axis=0)
    return (out['image_hash'], out['text_hash'],
            out['distill_i'], out['distill_t']), res


def kernel(**inputs):
    (ih, th, di, dtl), _ = run(inputs)
    return ih, th, di, dtl


# revision 34
# speedup vs baseline: 1.0488x; 1.0021x over previous
"""Trainium2 Bass kernel for nn_HashingModel (retrieval_knn).

Sharding: data-parallel over batch B across 8 cores (256 rows each).

v2 design (vs v1's K/V AllGather):
- Cross-batch MHA needs K/V for all 2048 keys. Every core already holds the
  full prompt table in DRAM, so we AllGather only the argmax *indices*
  (1 KB vs 512 KB), gather all 2048 prompt rows locally, and project K/V
  for the full batch on every core. The extra ~30us of replicated matmul
  per modality replaces 47-84us AllGather stalls that also tripped the
  PE activity throttle (HAM K=4/8) for whole phases.
- Similarity+argmax in fp32 (argmax must match the reference exactly;
  fp32 matmul measured at 2 cyc/row on HW). Sim drains on the Scalar
  engine so the Vector engine is free for argmax.
- K-proj bias dropped (softmax row-shift invariant; zero in practice),
  V-proj bias folded into the value table drain.
- MHA score/exp/AV loop is exp(Scalar)-paced; PE bubbles are filled by
  interleaving the other modality's K/V projection and W1*x precompute.
- Final Wc layer runs transposed (weights stationary, batch moving,
  N=512 instead of N=64) + PE transpose before the l2norm chain.
- All heavy matmuls bf16; weights pre-transposed host-side.

Self-contained: hardcoded shapes, no file reads.
"""
import sys
import numpy as np

sys.path.insert(0, '/opt/trn_rl_repo')

import ml_dtypes
from concourse import bass, bacc, tile, mybir
from concourse.bass_utils import run_bass_kernel_spmd
from concourse.masks import make_identity

dt = mybir.dt
BF16 = ml_dtypes.bfloat16
AF = mybir.ActivationFunctionType

FULL = dict(NC=8, B=2048, E=512, P=4096, H=8, HD=64, HID=4096, BIT=64)


def _cfg(NC, B, E, P, H, HD, HID, BIT):
    c = dict(NC=NC, B=B, E=E, P=P, H=H, HD=HD, HID=HID, BIT=BIT)
    c['BS'] = B // NC          # batch shard per core
    c['E2'] = 2 * E            # MLP input dim
    c['KT_E'] = E // 128       # k-tiles over E
    c['KT_E2'] = 2 * E // 128
    c['NT_HID'] = HID // 128
    c['LT'] = c['BS'] // 128   # l-tiles per shard
    c['ST'] = B // 128         # s-tiles over full batch
    c['PC'] = P // 512         # prompt chunks for sim
    c['B2'] = 2 * c['BS']      # MLP free dim (fi|ft)
    return c


def build_nc(cfg, n_cores):
    import os
    F_FILL = os.environ.get('F_FILL', '1') == '1'
    C = cfg
    NC = n_cores
    E, P, H, HD, HID, BIT = C['E'], C['P'], C['H'], C['HD'], C['HID'], C['BIT']
    BS, E2 = C['BS'], C['E2']
    KT_E, KT_E2, NT_HID, LT, ST, PC, B2 = (C['KT_E'], C['KT_E2'], C['NT_HID'],
                                           C['LT'], C['ST'], C['PC'], C['B2'])
    HPT = 128 // HD            # heads per 128-partition tile (2)
    NHT = E // 128             # eo tiles (4)
    SEG = HD + 1               # vaug segment width (65)
    S = C['B']                 # full batch (attention keys)
    SC = S // 512              # 512-col chunks of S
    FR = S // 16               # wrap16 columns for gather indices

    nc = bacc.Bacc("TRN2", target_bir_lowering=False, debug=False,
                   num_devices=NC)

    mods = ['i', 't']
    inp = {}

    def din(name, shape, d):
        inp[name] = nc.dram_tensor(name, shape, d, kind="ExternalInput")

    # weight layouts put each SBUF partition's data contiguous in DRAM so
    # every weight-block DMA is one large descriptor per partition
    for m in mods:
        din(f'xT_{m}', [E, BS], dt.float32)
        for w in ['wqT', 'wkT', 'wvT', 'woT']:
            din(f'{w}_{m}', [128, KT_E, E], dt.bfloat16)
        din(f'bq_{m}', [E], dt.float32)
        din(f'bo_{m}', [E], dt.float32)
        din(f'bv_{m}', [1, E], dt.bfloat16)
    din('promptsT', [E, P], dt.float32)
    din('prompts_bf', [P, E], dt.bfloat16)
    for M in ['img', 'txt']:
        din(f'w1T_{M}', [NT_HID, 128, KT_E2, 128], dt.bfloat16)
        din(f'w2T_{M}', [NT_HID, 128, NT_HID, 128], dt.bfloat16)
        din(f'wcT_{M}', [128, NT_HID, BIT], dt.bfloat16)
        din(f'b1_{M}', [HID], dt.float32)
        din(f'b2_{M}', [HID], dt.float32)
        din(f'bc_{M}', [1, BIT], dt.bfloat16)

    outs = {}
    for name in ['image_hash', 'text_hash', 'distill_i', 'distill_t']:
        outs[name] = nc.dram_tensor(name, [BS, BIT], dt.float32,
                                    kind="ExternalOutput")

    idx_in = {m: nc.dram_tensor(f'idx_in_{m}', [BS], dt.uint32) for m in mods}
    idx_out = {m: nc.dram_tensor(f'idx_out_{m}', [NC * BS], dt.uint32,
                                 addr_space="Shared") for m in mods}
    h1x_dram = {M: nc.dram_tensor(f'h1x_{M}', [NT_HID, 128, B2], dt.bfloat16)
                for M in ['img', 'txt']}

    with tile.TileContext(nc) as tc:
        with tc.tile_pool(name="persist", bufs=1) as pp:
            xTbf = {m: pp.tile([128, KT_E, BS], dt.bfloat16, tag=f'xTbf{m}',
                               name=f'xTbf{m}') for m in mods}
            inT = pp.tile([128, KT_E2, B2], dt.bfloat16, tag='inT')
            ones512 = pp.tile([1, 512], dt.bfloat16, tag='ones')
            nc.vector.memset(ones512[:], 1.0)
            ident = pp.tile([128, 128], dt.float32, tag='ident')
            make_identity(nc, ident[:])

            # weights + small tensors that live through phases A-C.
            # Tiles are allocated here; their DMAs are issued inside phase A
            # AFTER the sim inputs so the first matmul isn't queued behind
            # 4 MB of attention weights.
            with tc.tile_pool(name="wts", bufs=1) as wt:
                wk, wv, wq, wo, bvb, bqc, boc = {}, {}, {}, {}, {}, {}, {}
                bvr = {}
                for m in mods:
                    wk[m] = wt.tile([128, KT_E, E], dt.bfloat16, tag=f'wk{m}',
                                    name=f'wk{m}')
                    wv[m] = wt.tile([128, KT_E, E], dt.bfloat16, tag=f'wv{m}',
                                    name=f'wv{m}')
                    wq[m] = wt.tile([128, KT_E, E], dt.bfloat16, tag=f'wq{m}',
                                    name=f'wq{m}')
                    wo[m] = wt.tile([128, KT_E, E], dt.bfloat16, tag=f'wo{m}',
                                    name=f'wo{m}')
                    bvr[m] = wt.tile([1, E], dt.bfloat16, tag=f'bvr{m}',
                                     name=f'bvr{m}')
                    bvb[m] = wt.tile([128, E], dt.bfloat16, tag=f'bvb{m}',
                                     name=f'bvb{m}')
                    bqc[m] = wt.tile([128, NHT], dt.float32, tag=f'bq{m}',
                                     name=f'bq{m}')
                    boc[m] = wt.tile([128, NHT], dt.float32, tag=f'bo{m}',
                                     name=f'bo{m}')

                def load_attn_weights():
                    for m in mods:
                        nc.sync.dma_start(wk[m][:], inp[f'wkT_{m}'].ap())
                        nc.sync.dma_start(wv[m][:], inp[f'wvT_{m}'].ap())
                        nc.sync.dma_start(wq[m][:], inp[f'wqT_{m}'].ap())
                        nc.sync.dma_start(wo[m][:], inp[f'woT_{m}'].ap())
                        nc.sync.dma_start(bvr[m][:], inp[f'bv_{m}'].ap())
                        nc.gpsimd.partition_broadcast(bvb[m][:], bvr[m][:])
                        nc.sync.dma_start(
                            bqc[m][:],
                            inp[f'bq_{m}'].ap().rearrange("(t p) -> p t", p=128))
                        nc.sync.dma_start(
                            boc[m][:],
                            inp[f'bo_{m}'].ap().rearrange("(t p) -> p t", p=128))
                # gathered prompt rows, chunked so each 256-idx sub-gather
                # writes a contiguous [128, KT_E, 256] block
                JC = S // 256
                rmT = {m: wt.tile([128, JC, KT_E, 256], dt.bfloat16,
                                  tag=f'rmT{m}', name=f'rmT{m}') for m in mods}

                # ======== Phase A: sim -> argmax -> idx AllGather ->
                # local gather of all 2048 prompt rows per modality ========
                with (
                    tc.tile_pool(name="phA", bufs=1) as ap_,
                    tc.tile_pool(name="simbuf", bufs=2) as simp,
                    tc.tile_pool(name="smA", bufs=4) as sp,
                    tc.tile_pool(name="psA", bufs=4, space="PSUM") as psA,
                ):
                    xT32 = {m: ap_.tile([128, KT_E, BS], dt.float32,
                                        tag=f'xT32{m}', name=f'xT32{m}')
                            for m in mods}
                    for mi, m in enumerate(mods):
                        nc.sync.dma_start(
                            xT32[m][:],
                            inp[f'xT_{m}'].ap().rearrange("(k p) b -> p k b", p=128))
                        nc.vector.tensor_copy(xTbf[m][:], xT32[m][:])
                        nc.vector.tensor_copy(
                            inT[:, 0:KT_E, mi * BS:(mi + 1) * BS], xTbf[m][:])
                    prT = ap_.tile([128, KT_E, P], dt.float32, tag='promptsT')
                    prsrc = inp['promptsT'].ap().rearrange(
                        "(k p) (h n) -> k p h n", p=128, h=2)
                    for hf in range(2):
                        for k in range(KT_E):
                            nc.sync.dma_start(
                                prT[:, k, hf * (P // 2):(hf + 1) * (P // 2)],
                                prsrc[k, :, hf])
                    load_attn_weights()

                    for m in mods:
                        sims = []
                        for lt in range(LT):
                            sim = simp.tile([128, P], dt.float32, tag='sim')
                            sims.append(sim)
                            for pc in range(PC):
                                ps = psA.tile([128, 512], dt.float32,
                                              tag='ps_sim')
                                for k in range(KT_E):
                                    nc.tensor.matmul(
                                        ps[:],
                                        xT32[m][:, k, lt * 128:(lt + 1) * 128],
                                        prT[:, k, pc * 512:(pc + 1) * 512],
                                        start=(k == 0), stop=(k == KT_E - 1))
                                # drain on Scalar engine: DVE stays free for
                                # argmax, PE for sim
                                nc.scalar.copy(sim[:, pc * 512:(pc + 1) * 512],
                                               ps[:])
                            m8 = sp.tile([128, 8], dt.float32, tag='m8')
                            i8 = sp.tile([128, 8], dt.uint32, tag='i8')
                            nc.vector.max(m8[:], sim[:])
                            nc.vector.max_index(i8[:], m8[:], sim[:])
                            # ACT-issued DMA: keeps the index write off the
                            # congested weight-streaming queue
                            nc.scalar.dma_start(
                                idx_in[m].ap()[lt * 128:(lt + 1) * 128],
                                i8[:, 0:1])
                        if NC > 1:
                            nc.gpsimd.collective_compute(
                                "AllGather", mybir.AluOpType.bypass,
                                replica_groups=[list(range(NC))],
                                ins=[idx_in[m][:]], outs=[idx_out[m][:]])
                        else:
                            nc.scalar.dma_start(idx_out[m].ap()[0:BS],
                                                idx_in[m].ap())
                        # wrap16 + replicate + int16 for the gpsimd gather
                        i32 = sp.tile([128, FR], dt.uint32, tag='i32g')
                        for a in range(2):
                            nc.scalar.dma_start(
                                i32[16 * a:16 * (a + 1), :],
                                idx_out[m].ap().rearrange("(f p) -> p f", p=16))
                        nc.vector.tensor_copy(i32[32:64, :], i32[0:32, :])
                        nc.vector.tensor_copy(i32[64:128, :], i32[0:64, :])
                        ix16 = sp.tile([128, FR], dt.int16, tag='i16g')
                        nc.vector.tensor_copy(ix16[:], i32[:])
                        for j in range(S // 256):
                            nc.gpsimd.dma_gather(
                                rmT[m][:, j], inp['prompts_bf'].ap(),
                                ix16[:, 16 * j:16 * (j + 1)],
                                num_idxs=256, num_idxs_reg=256, elem_size=E,
                                transpose=True)

                # ======== Phase C: K/V proj (full batch, local) + MHA.
                # The other modality's K/V projection and the W1*x
                # precompute fill the exp-paced PE bubbles. ========
                with (
                    tc.tile_pool(name="phC", bufs=1) as cp_,
                    tc.tile_pool(name="w1xp", bufs=3) as w1p,
                    tc.tile_pool(name="expp", bufs=2) as ep,
                    tc.tile_pool(name="smC", bufs=2) as spC,
                    tc.tile_pool(name="smZ", bufs=1) as spZ,
                    tc.tile_pool(name="psB", bufs=2, space="PSUM") as psB,
                    tc.tile_pool(name="psS", bufs=2, space="PSUM") as psS,
                    tc.tile_pool(name="psO", bufs=1, space="PSUM") as psO,
                ):
                    kpT = {m: cp_.tile([128, NHT, S], dt.bfloat16,
                                       tag=f'kpT{m}', name=f'kpT{m}')
                           for m in mods}
                    vaug = {m: cp_.tile([128, ST, H * SEG], dt.bfloat16,
                                        tag=f'vaug{m}', name=f'vaug{m}')
                            for m in mods}
                    qpT = {m: cp_.tile([128, NHT, BS], dt.bfloat16,
                                       tag=f'qpT{m}', name=f'qpT{m}')
                           for m in mods}
                    for m in mods:
                        # only the per-segment ones column (index HD) needs
                        # init: data columns are written by the V-proj drain
                        nc.vector.memset(
                            vaug[m][:].rearrange("p st (h s) -> p st h s",
                                                 h=H)[:, :, :, HD], 1.0)

                    b1x = {}
                    for M in ['img', 'txt']:
                        b1x[M] = spC.tile([128, NT_HID], dt.float32,
                                          tag=f'b1x{M}', name=f'b1x{M}')
                        nc.sync.dma_start(
                            b1x[M][:],
                            inp[f'b1_{M}'].ap().rearrange("(t p) -> p t", p=128))

                    def kv_chunk(m, j):
                        # j in [0, 2*SC): first SC chunks: kp eo-groups;
                        # rest: vaug 4-st groups
                        if j < SC:
                            eo = j
                            for sc in range(SC):
                                ps = psB.tile([128, 512], dt.float32, tag='ps_kv')
                                for k in range(KT_E):
                                    nc.tensor.matmul(
                                        ps[:], wk[m][:, k, eo * 128:(eo + 1) * 128],
                                        rmT[m][:, 2 * sc:2 * sc + 2, k, :],
                                        start=(k == 0), stop=(k == KT_E - 1))
                                nc.vector.tensor_copy(
                                    kpT[m][:, eo, sc * 512:(sc + 1) * 512], ps[:])
                        else:
                            for st in range((j - SC) * 4, (j - SC) * 4 + 4):
                                ps = psB.tile([128, E], dt.float32, tag='ps_kv')
                                for k in range(KT_E):
                                    nc.tensor.matmul(
                                        ps[:],
                                        rmT[m][:, st // 2, k,
                                               (st % 2) * 128:(st % 2) * 128 + 128],
                                        wv[m][:, k, :],
                                        start=(k == 0), stop=(k == KT_E - 1))
                                nc.vector.tensor_tensor(
                                    out=vaug[m][:, st, :].rearrange(
                                        "p (h s) -> p h s", h=H)[:, :, 0:HD],
                                    in0=ps[:].rearrange("p (h d) -> p h d", h=H),
                                    in1=bvb[m][:].rearrange("p (h d) -> p h d", h=H),
                                    op=mybir.AluOpType.add)

                    def qproj(m):
                        for eo in range(NHT):
                            psf = psS.tile([128, 2 * BS], dt.float32,
                                           tag='ps_s', name='ps_qf')
                            ps = psf[:, 0:BS]
                            for k in range(KT_E):
                                nc.tensor.matmul(
                                    ps[:], wq[m][:, k, eo * 128:(eo + 1) * 128],
                                    xTbf[m][:, k, :], start=(k == 0),
                                    stop=(k == KT_E - 1))
                            nc.vector.tensor_scalar_add(qpT[m][:, eo, :], ps[:],
                                                        bqc[m][:, eo:eo + 1])

                    # W1*x precompute chunks (PE bubble filler; DRAM staging).
                    # Only chunks actually consumed as fillers are
                    # precomputed; the rest run full-K in phase D.
                    h1x_jobs = [(M, ht) for M in ['img', 'txt']
                                for ht in range(NT_HID)]
                    h1x_pos = [0]
                    h1x_done = set()

                    def h1x_chunk(n=1):
                        for _ in range(n):
                            if h1x_pos[0] >= len(h1x_jobs):
                                return
                            M, ht = h1x_jobs[h1x_pos[0]]
                            h1x_pos[0] += 1
                            h1x_done.add((M, ht))
                            wblk = w1p.tile([128, KT_E, 128], dt.bfloat16,
                                            tag='w1xblk')
                            nc.sync.dma_start(
                                wblk[:], inp[f'w1T_{M}'].ap()[ht, :, 0:KT_E])
                            ps = psB.tile([128, B2], dt.float32, tag='ps_kv')
                            for k in range(KT_E):
                                nc.tensor.matmul(ps[:], wblk[:, k, :],
                                                 inT[:, k, :],
                                                 start=(k == 0),
                                                 stop=(k == KT_E - 1))
                            hx = w1p.tile([128, B2], dt.bfloat16, tag='h1x_sb')
                            nc.vector.tensor_scalar_add(hx[:], ps[:],
                                                        b1x[M][:, ht:ht + 1])
                            nc.sync.dma_start(h1x_dram[M].ap()[ht], hx[:])

                    def mha_loop(m, filler):
                        pso = [psO.tile([SEG, HPT * BS], dt.float32,
                                        tag=f'pso{g}', name=f'pso{g}')
                               for g in range(H // HPT)]
                        for st2 in range(0, ST, 2):
                            ex = ep.tile([128, H, 2 * BS], dt.bfloat16,
                                         tag='expT')
                            for g in range(H // HPT):
                                for hh in range(HPT):
                                    h = g * HPT + hh
                                    hb = hh * HD
                                    pss = psS.tile([128, 2 * BS], dt.float32,
                                                   tag='ps_s')
                                    for sj in range(2):
                                        st = st2 + sj
                                        nc.tensor.matmul(
                                            pss[:, sj * BS:(sj + 1) * BS],
                                            kpT[m][hb:hb + HD, g,
                                                   st * 128:(st + 1) * 128],
                                            qpT[m][hb:hb + HD, g, :],
                                            start=True, stop=True,
                                            skip_group_check=True)
                                    nc.scalar.activation(
                                        ex[:, h, :], pss[:], AF.Exp,
                                        bias=0.0,
                                        scale=float(1.0 / np.sqrt(HD)))
                                    for sj in range(2):
                                        st = st2 + sj
                                        nc.tensor.matmul(
                                            pso[g][:, hh * BS:(hh + 1) * BS],
                                            vaug[m][:, st, h * SEG:(h + 1) * SEG],
                                            ex[:, h, sj * BS:(sj + 1) * BS],
                                            start=(st == 0), stop=(st == ST - 1),
                                            skip_group_check=True)
                            filler(st2)
                        return pso

                    def mha_finish(m, mi, pso):
                        # z-row extract (Scalar), broadcast (GpSimd), then a
                        # wide reciprocal (partition-parallel, fast on DVE)
                        zr = spZ.tile([1, H * BS], dt.float32, tag='zr')
                        for h in range(H):
                            nc.scalar.copy(
                                zr[0:1, h * BS:(h + 1) * BS],
                                pso[h // HPT][HD:HD + 1,
                                              (h % HPT) * BS:(h % HPT + 1) * BS])
                        zb = spZ.tile([HD, H * BS], dt.float32, tag='zb')
                        nc.gpsimd.partition_broadcast(zb[:], zr[:])
                        zbi = spZ.tile([HD, H * BS], dt.float32, tag='zbi')
                        nc.vector.reciprocal(zbi[:], zb[:])
                        aoT = cp_.tile([128, NHT, BS], dt.bfloat16, tag='aoT',
                                       name=f'aoT{m}')
                        for h in range(H):
                            nc.vector.tensor_tensor(
                                out=aoT[(h % HPT) * HD:(h % HPT + 1) * HD,
                                        h // HPT, :],
                                in0=pso[h // HPT][0:HD,
                                                  (h % HPT) * BS:(h % HPT + 1) * BS],
                                in1=zbi[:, h * BS:(h + 1) * BS],
                                op=mybir.AluOpType.mult)
                        # out projection -> inT enh rows
                        for eo in range(NHT):
                            psf = psS.tile([128, 2 * BS], dt.float32,
                                           tag='ps_s', name='ps_of')
                            ps = psf[:, 0:BS]
                            for k in range(KT_E):
                                nc.tensor.matmul(
                                    ps[:], wo[m][:, k, eo * 128:(eo + 1) * 128],
                                    aoT[:, k, :], start=(k == 0),
                                    stop=(k == KT_E - 1))
                            nc.vector.tensor_scalar_add(
                                inT[:, KT_E + eo, mi * BS:(mi + 1) * BS], ps[:],
                                boc[m][:, eo:eo + 1])

                    # ---- phase C schedule ----
                    # h1x chunks before kvproj_i cover the idx-AllGather +
                    # gather latency; the 't' K/V projection runs between the
                    # two attention loops (covering modality-i's z-chain);
                    # out-projections are deferred so the PE never waits on
                    # the Vector/Scalar z work.
                    if F_FILL:
                        h1x_chunk(26)
                    for j in range(2 * SC):
                        kv_chunk('i', j)
                    qproj('i')
                    if F_FILL:
                        pso_i = mha_loop('i', lambda st2: h1x_chunk(2))
                        h1x_chunk(6)
                        for j in range(2 * SC):
                            kv_chunk('t', j)
                        qproj('t')
                        mha_finish('i', 0, pso_i)
                        pso_t = mha_loop('t', lambda st2: h1x_chunk(2))
                        mha_finish('t', 1, pso_t)
                    else:
                        pso_i = mha_loop('i', lambda st2: None)
                        mha_finish('i', 0, pso_i)
                        for j in range(2 * SC):
                            kv_chunk('t', j)
                        qproj('t')
                        pso_t = mha_loop('t', lambda st2: None)
                        mha_finish('t', 1, pso_t)

            # ======== Phase D: the four MLPs (two weight passes) ========
            with (
                tc.tile_pool(name="phD", bufs=1) as dp_,
                tc.tile_pool(name="w1s", bufs=3) as wp,
                tc.tile_pool(name="w2s", bufs=3) as w2p,
                tc.tile_pool(name="smD", bufs=2) as spD,
                tc.tile_pool(name="psD", bufs=4, space="PSUM") as psD,
                tc.tile_pool(name="psW", bufs=2, space="PSUM") as psW,
                tc.tile_pool(name="psT", bufs=2, space="PSUM") as psT,
            ):
                h1T = dp_.tile([128, NT_HID, B2], dt.bfloat16, tag='h1T')
                h2T = dp_.tile([128, NT_HID, B2], dt.bfloat16, tag='h2T')
                out_map = {'img': ['image_hash', 'distill_i'],
                           'txt': ['distill_t', 'text_hash']}
                for M in ['img', 'txt']:
                    b2 = spD.tile([128, NT_HID], dt.float32, tag='b2')
                    bc = spD.tile([1, BIT], dt.bfloat16, tag='bc')
                    nc.sync.dma_start(
                        b2[:], inp[f'b2_{M}'].ap().rearrange("(t p) -> p t", p=128))
                    nc.sync.dma_start(bc[:], inp[f'bc_{M}'].ap())

                    b1f = spD.tile([128, NT_HID], dt.float32, tag='b1f')
                    nc.sync.dma_start(
                        b1f[:], inp[f'b1_{M}'].ap().rearrange("(t p) -> p t", p=128))
                    for ht in range(NT_HID):
                        if (M, ht) in h1x_done:
                            wblk = wp.tile([128, KT_E, 128], dt.bfloat16,
                                           tag='w1blk')
                            nc.sync.dma_start(
                                wblk[:], inp[f'w1T_{M}'].ap()[ht, :, KT_E:KT_E2])
                            hx = wp.tile([128, B2], dt.bfloat16, tag='h1x_ld')
                            nc.sync.dma_start(hx[:], h1x_dram[M].ap()[ht])
                            ps = psD.tile([128, B2], dt.float32, tag='ps_h12')
                            for k in range(KT_E):
                                nc.tensor.matmul(ps[:], wblk[:, k, :],
                                                 inT[:, KT_E + k, :],
                                                 start=(k == 0),
                                                 stop=(k == KT_E - 1))
                            hpre = wp.tile([128, B2], dt.float32, tag='h1pre')
                            nc.vector.tensor_tensor(out=hpre[:], in0=ps[:],
                                                    in1=hx[:],
                                                    op=mybir.AluOpType.add)
                            nc.vector.tensor_scalar_max(h1T[:, ht, :], hpre[:],
                                                        0.0)
                        else:
                            wblk = wp.tile([128, KT_E2, 128], dt.bfloat16,
                                           tag='w1blkf', name='w1blkf')
                            nc.sync.dma_start(
                                wblk[:], inp[f'w1T_{M}'].ap()[ht])
                            ps = psD.tile([128, B2], dt.float32, tag='ps_h12')
                            for k in range(KT_E2):
                                nc.tensor.matmul(ps[:], wblk[:, k, :],
                                                 inT[:, k, :],
                                                 start=(k == 0),
                                                 stop=(k == KT_E2 - 1))
                            nc.vector.tensor_scalar(
                                h1T[:, ht, :], ps[:], b1f[:, ht:ht + 1], 0.0,
                                op0=mybir.AluOpType.add, op1=mybir.AluOpType.max)

                    for ht in range(NT_HID):
                        wblk = w2p.tile([128, NT_HID, 128], dt.bfloat16,
                                        tag='w2blk')
                        nc.sync.dma_start(
                            wblk[:], inp[f'w2T_{M}'].ap()[ht])
                        ps = psD.tile([128, B2], dt.float32, tag='ps_h12')
                        for k in range(NT_HID):
                            nc.tensor.matmul(ps[:], wblk[:, k, :], h1T[:, k, :],
                                             start=(k == 0), stop=(k == NT_HID - 1))
                        nc.vector.tensor_scalar(
                            h2T[:, ht, :], ps[:], b2[:, ht:ht + 1], 0.0,
                            op0=mybir.AluOpType.add, op1=mybir.AluOpType.max)

                    # Wc transposed: weights stationary, batch moving (N=512)
                    wc = dp_.tile([128, NT_HID, BIT], dt.bfloat16, tag='wc')
                    nc.sync.dma_start(wc[:], inp[f'wcT_{M}'].ap())
                    psw = psW.tile([BIT, B2], dt.float32, tag='ps_wc')
                    for k in range(NT_HID):
                        nc.tensor.matmul(psw[:], wc[:, k, :], h2T[:, k, :],
                                         start=(k == 0), stop=False)
                    nc.tensor.matmul(psw[:], bc[:], ones512[:],
                                     start=False, stop=True)
                    h3T = spD.tile([BIT, B2], dt.float32, tag='h3T')
                    nc.scalar.copy(h3T[:], psw[:])
                    for j in range(B2 // 128):
                        pst = psT.tile([128, BIT], dt.float32, tag='ps_t')
                        nc.tensor.transpose(pst[:], h3T[:, j * 128:(j + 1) * 128],
                                            ident[0:BIT, 0:BIT])
                        sq = spD.tile([128, BIT], dt.float32, tag='sq')
                        ss = spD.tile([128, 1], dt.float32, tag='ss')
                        nc.scalar.activation(sq[:], pst[:], AF.Square,
                                             accum_out=ss[:])
                        rs = spD.tile([128, 1], dt.float32, tag='rs')
                        nc.vector.reciprocal(rs[:], ss[:])
                        rsq = spD.tile([128, 1], dt.float32, tag='rsq')
                        nc.scalar.sqrt(rsq[:], rs[:])
                        h3 = spD.tile([128, BIT], dt.float32, tag='h3')
                        nc.vector.tensor_scalar_mul(h3[:], pst[:], rsq[:])
                        oname = out_map[M][j // LT]
                        row = (j % LT) * 128
                        nc.sync.dma_start(outs[oname].ap()[row:row + 128, :],
                                          h3[:])

    nc.compile()
    return nc


def _prep_in_maps(cfg, n_cores, image_feature, text_feature, prompts,
                  img_in_w, img_in_b, img_out_w, img_out_b,
                  txt_in_w, txt_in_b, txt_out_w, txt_out_b,
                  img_W1, img_b1, img_W2, img_b2, img_Wc, img_bc,
                  txt_W1, txt_b1, txt_W2, txt_b2, txt_Wc, txt_bc):
    C = cfg
    E, P, BIT, BS = C['E'], C['P'], C['BIT'], C['BS']
    NT_HID, KT_E2 = C['NT_HID'], C['KT_E2']

    def bt(x):
        return np.ascontiguousarray(np.asarray(x).astype(BF16))

    common = {}
    common['promptsT'] = np.ascontiguousarray(prompts.T.astype(np.float32))
    common['prompts_bf'] = bt(prompts)

    KT_E = E // 128

    def kpn(w):  # [E, N] -> [128, KT_E, N] (partition-contiguous in DRAM)
        w = np.asarray(w).astype(BF16)
        return np.ascontiguousarray(
            w.reshape(KT_E, 128, w.shape[1]).transpose(1, 0, 2))

    for m, in_w, in_b, out_w, out_b in [
            ('i', img_in_w, img_in_b, img_out_w, img_out_b),
            ('t', txt_in_w, txt_in_b, txt_out_w, txt_out_b)]:
        common[f'wqT_{m}'] = kpn(in_w[:E].T)
        common[f'wkT_{m}'] = kpn(in_w[E:2 * E].T)
        common[f'wvT_{m}'] = kpn(in_w[2 * E:].T)
        common[f'woT_{m}'] = kpn(out_w.T)
        common[f'bq_{m}'] = np.ascontiguousarray(in_b[:E].astype(np.float32))
        common[f'bv_{m}'] = bt(in_b[2 * E:][None, :])
        common[f'bo_{m}'] = np.ascontiguousarray(out_b.astype(np.float32))

    for M, W1, b1, W2, b2, Wc, bc in [
            ('img', img_W1, img_b1, img_W2, img_b2, img_Wc, img_bc),
            ('txt', txt_W1, txt_b1, txt_W2, txt_b2, txt_Wc, txt_bc)]:
        w1t = np.asarray(W1).T.astype(BF16)      # [2E, HID]
        common[f'w1T_{M}'] = np.ascontiguousarray(
            w1t.reshape(KT_E2, 128, NT_HID, 128).transpose(2, 1, 0, 3))
        w2t = np.asarray(W2).T.astype(BF16)      # [HID, HID]
        common[f'w2T_{M}'] = np.ascontiguousarray(
            w2t.reshape(NT_HID, 128, NT_HID, 128).transpose(2, 1, 0, 3))
        wct = np.asarray(Wc).T.astype(BF16)      # [HID, BIT]
        common[f'wcT_{M}'] = np.ascontiguousarray(
            wct.reshape(NT_HID, 128, BIT).transpose(1, 0, 2))
        common[f'b1_{M}'] = np.ascontiguousarray(b1.astype(np.float32))
        common[f'b2_{M}'] = np.ascontiguousarray(b2.astype(np.float32))
        common[f'bc_{M}'] = bt(np.asarray(bc)[None, :])

    xTi = np.asarray(image_feature).T.astype(np.float32)
    xTt = np.asarray(text_feature).T.astype(np.float32)
    in_maps = []
    for c in range(n_cores):
        im = dict(common)
        im['xT_i'] = np.ascontiguousarray(xTi[:, c * BS:(c + 1) * BS])
        im['xT_t'] = np.ascontiguousarray(xTt[:, c * BS:(c + 1) * BS])
        in_maps.append(im)
    return in_maps


_NC_CACHE = {}


def _get_nc(cfg, n_cores):
    key = (tuple(sorted(cfg.items())), n_cores)
    if key not in _NC_CACHE:
        _NC_CACHE[key] = build_nc(cfg, n_cores)
    return _NC_CACHE[key]


def run(inputs, cfg=None, n_cores=None, trace=False):
    cfg = cfg or _cfg(**FULL)
    n_cores = n_cores or cfg['NC']
    nc = _get_nc(cfg, n_cores)
    in_maps = _prep_in_maps(cfg, n_cores, **{
        k: np.asarray(v) for k, v in inputs.items() if k != 'iteration'})
    res = run_bass_kernel_spmd(nc, in_maps, list(range(n_cores)), trace=trace)
    out = {}
    for name in ['image_hash', 'text_hash', 'distill_i', 'distill_t']:
        out[name] = np.concatenate(
            [res.results[c][name] for c in range(n_cores)], axis=0)
    return (out['image_hash'], out['text_hash'],
            out['distill_i'], out['distill_t']), res


def kernel(**inputs):
    (ih, th, di, dtl), _ = run(inputs)
    return ih, th, di, dtl
